# revision 1
# baseline (speedup 1.0000x reference)
"""Multi-head causal attention (B=2, T=2048, C=1024, H=16, D=64) on 8 trn2 cores.

Sharding: core c -> batch b = c//4, head group g = c%4 (4 heads each),
Megatron-style: QKV column-parallel, proj row-parallel. Partial outputs are
summed on the host; bk is softmax-invariant and dropped, bv/bp fold into a
host-side constant. All matmul operands are bf16 (host-cast), accumulation
and softmax stay fp32.

Device kernel (per core):
  A = x[b].T                       [1024, 2048]  host-transposed + repacked
  Q^T (+bq) / K^T = W.T @ A        [256, 2048]   channels on partitions
  V = A.T @ Wv_loc.T               [2048, 4*(64+1)]  natural layout, a ones
                                   column per head for softmax denominators
  per 512-wide q chunk qj, head h, 128-key chunk kc (diagonal chunks first,
  S emitted 3 ahead of PV so the in-order PE queue never waits on exp):
     S^T[k,q] = K_h^T.T @ Q_h^T    PSUM, trimmed to the causal q-suffix
     P^T = exp(0.125*S^T)          ACT runs exclusively Exp (no table swaps)
     diagonal chunks masked on GpSimd via affine_select
     PV~[65,*] += V~_h[kc].T @ P^T     row 64 accumulates the denominator l
     out^T = PV[0:64] * bcast(1/l)     approx-recip + gpsimd partition_broadcast
  Y = attn-out^T.T @ Wp_loc.T      [2048, 1024]  partial, proj interleaved
                                   per q-chunk, summed on host
"""

import sys

sys.path.insert(0, "/opt/trn_rl_repo")

import numpy as np
import ml_dtypes

NP_DT = ml_dtypes.bfloat16

import concourse.bass as bass  # noqa: F401
import concourse.mybir as mybir
import concourse.tile as tile
from concourse import bacc
from concourse.bass_utils import run_bass_kernel_spmd

N_CORES = 8
B, T, C = 2, 2048, 1024
H, D = 16, 64
H_LOC = 4              # heads per core
OL = H_LOC * D         # local channels = 256
CQ = 512               # PSUM-bank q chunk
CW = 1024              # exp window (2 PSUM banks)
CK = 128               # k chunk (partition dim)
NW = T // CW           # 2
NT = T // 128          # 16
KC = C // 128          # 8 contraction chunks for QKV

f32 = mybir.dt.float32
f32r = mybir.dt.float32r
bf16 = mybir.dt.bfloat16
DT = bf16  # matmul operand dtype

_COMPILED = None


def _build():
    nc = bacc.Bacc("TRN2", debug=False, num_devices=N_CORES)

    A = nc.dram_tensor("A", [2 * C, CW], DT, kind="ExternalInput").ap()
    Wqkv = nc.dram_tensor("Wqkv", [C, 3 * OL], DT, kind="ExternalInput").ap()
    WpT = nc.dram_tensor("WpT", [OL, C], DT, kind="ExternalInput").ap()
    BQ = nc.dram_tensor("BQ", [OL, 1], f32, kind="ExternalInput").ap()
    Y = nc.dram_tensor("Y", [T, C], f32, kind="ExternalOutput").ap()

    Exp = mybir.ActivationFunctionType.Exp

    with tile.TileContext(nc) as tc:
        with tc.tile_pool(name="sbuf", bufs=1) as pool, \
             tc.tile_pool(name="work", bufs=1) as wpool, \
             tc.tile_pool(name="psum", bufs=1, space="PSUM") as psum:

            # ---- resident inputs (piece-contiguous loads, weights first) ----
            a_t, w_t = [], []
            for kc in range(KC):
                at = pool.tile([128, T], DT, tag=f"A{kc}", name=f"a{kc}")
                a_t.append(at)
                wt = pool.tile([128, 3 * OL], DT, tag=f"W{kc}", name=f"w{kc}")
                w_t.append(wt)
            # A is host-repacked so block (kc, piece) = rows
            # (kc*4+piece)*128..+128 is one contiguous 128KB read. Load
            # w[kc] + piece-0 of a[kc] interleaved so the first QKV
            # accumulation chain (needs all kc) completes earliest.
            for kc in range(KC):
                nc.sync.dma_start(w_t[kc][:], Wqkv[kc * 128:(kc + 1) * 128, :])
                blk = kc * 2 * 128
                nc.sync.dma_start(
                    a_t[kc][:, 0:CW],
                    A[blk:blk + 128, 0:CW])
            for kc in range(KC):
                blk = (kc * 2 + 1) * 128
                nc.sync.dma_start(
                    a_t[kc][:, CW:T],
                    A[blk:blk + 128, 0:CW])
            wp_t = []
            for kc in range(2):
                wp = pool.tile([128, C], DT, tag=f"WP{kc}", name=f"wp{kc}")
                nc.sync.dma_start(wp[:], WpT[kc * 128:(kc + 1) * 128, :])
                wp_t.append(wp)
            bq_t = []
            for m in range(2):
                bq = pool.tile([128, 1], f32, tag=f"BQ{m}", name=f"bq{m}")
                nc.sync.dma_start(bq[:], BQ[m * 128:(m + 1) * 128, :])
                bq_t.append(bq)
            col1 = pool.tile([128, 1], f32, tag="col1")
            nc.vector.memset(col1[:], 1.0)
            # warm the GpSimd ucode paths so the first real mask/broadcast
            # doesn't eat the cold-start cost mid-attention
            warm = wpool.tile([128, 8], f32, tag="warm")
            nc.vector.memset(warm[:], 1.0)
            nc.gpsimd.affine_select(
                out=warm[:], in_=warm[:],
                compare_op=mybir.AluOpType.is_ge, fill=0.0, base=0,
                pattern=[[1, 8]], channel_multiplier=-1)
            warm2 = wpool.tile([128, 8], f32, tag="warm2")
            nc.gpsimd.partition_broadcast(warm2[:], warm[0:1, :])

            # ---- persistent intermediates ----
            qt_sb = [pool.tile([128, T], DT, tag=f"QT{i}", name=f"qt{i}")
                     for i in range(2)]
            kt_sb = [pool.tile([128, T], DT, tag=f"KT{i}", name=f"kt{i}")
                     for i in range(2)]
            v_sb = [pool.tile([128, H_LOC * (D + 1)], DT, tag=f"V{i}",
                              name=f"v{i}") for i in range(NT)]
            ao_sb = [pool.tile([128, T], DT, tag=f"AO{i}", name=f"ao{i}")
                     for i in range(2)]

            # ---- phase 1a: Q^T, K^T (evict on DVE; ACT is Exp-only) ----
            for m in range(4):
                for n in range(T // CQ):
                    ps = psum.tile([128, CQ], f32, tag="prj", bufs=2, name="ps")
                    for kc in range(KC):
                        nc.tensor.matmul(
                            ps[:],
                            w_t[kc][:, m * 128:(m + 1) * 128],
                            a_t[kc][:, n * CQ:(n + 1) * CQ],
                            start=(kc == 0), stop=(kc == KC - 1))
                    if m < 2:
                        nc.vector.tensor_scalar_add(
                            qt_sb[m][:, n * CQ:(n + 1) * CQ], ps[:],
                            bq_t[m][:, 0:1])
                    else:
                        nc.vector.tensor_copy(
                            kt_sb[m - 2][:, n * CQ:(n + 1) * CQ], ps[:])

            # ---- phase 1b: V natural layout ----
            for tt in range(NT):
                ps = psum.tile([128, CQ], f32, tag="mm", bufs=4, name="psv")[:, 0:OL]
                for kc in range(KC):
                    nc.tensor.matmul(
                        ps[:],
                        a_t[kc][:, tt * 128:(tt + 1) * 128],
                        w_t[kc][:, 2 * OL:3 * OL],
                        start=(kc == 0), stop=(kc == KC - 1))
                # single strided copy for all 4 heads' V columns frees
                # the PSUM slot ~3x sooner than 4 serial copies
                nc.vector.tensor_copy(
                    v_sb[tt].rearrange("p (h x) -> p h x", x=D + 1)[:, :, 0:D],
                    ps.rearrange("p (h x) -> p h x", x=D))
                for h in range(H_LOC):
                    nc.vector.tensor_copy(
                        v_sb[tt][:, h * (D + 1) + D:(h + 1) * (D + 1)],
                        col1[:])

            # ---- phase 2+3: causal flash attention, proj interleaved ----
            # One GLOBAL software pipeline over all (qj, h) blocks: S/exp
            # emission runs LOOKAHEAD work-items ahead of the PV consumer
            # across block boundaries, so the in-order PE queue never
            # drains/refills between blocks. Diagonal chunks first per
            # block so the GpSimd mask latency hides behind other S work.
            blocks = []
            for qj in range(T // CQ):
                for h in range(H_LOC):
                    n_kc = (qj + 1) * (CQ // CK)
                    order = list(range(qj * 4, n_kc)) + list(range(0, qj * 4))
                    blocks.append((qj, h, order))
            flat = [(bi, j) for bi, (_, _, order) in enumerate(blocks)
                    for j in range(len(order))]
            LOOKAHEAD = 3
            pv_tiles = {}
            pts = {}

            def emit_s(idx):
                bi, j = flat[idx]
                qj, h, order = blocks[bi]
                kc = order[j]
                ht, hp, q0 = h // 2, (h % 2) * 64, qj * CQ
                # diagonal chunks only cover q >= kc*CK: trim the
                # S/exp/PV stream to the causally valid q suffix
                qoff = max(0, kc * CK - q0)   # 0/128/256/384
                width = CQ - qoff
                sp = psum.tile([128, CQ], f32, tag="mm", bufs=4)
                nc.tensor.matmul(
                    sp[:, 0:width],
                    kt_sb[ht][hp:hp + D, kc * CK:(kc + 1) * CK],
                    qt_sb[ht][hp:hp + D, q0 + qoff:q0 + CQ],
                    start=True, stop=True)
                pt = wpool.tile([128, CQ], DT, tag="pT", bufs=12)
                nc.scalar.activation(pt[:, 0:width], sp[:, 0:width],
                                     Exp, scale=1.0 / 8.0)
                if kc >= qj * 4:   # diagonal chunk: mask q < k
                    nc.gpsimd.affine_select(
                        out=pt[:, 0:width], in_=pt[:, 0:width],
                        compare_op=mybir.AluOpType.is_ge,
                        fill=0.0, base=0,
                        pattern=[[1, width]], channel_multiplier=-1)
                pts[(bi, kc)] = (pt, qoff, width)

            for idx in range(min(LOOKAHEAD, len(flat))):
                emit_s(idx)
            for i, (bi, j) in enumerate(flat):
                if i + LOOKAHEAD < len(flat):
                    emit_s(i + LOOKAHEAD)
                qj, h, order = blocks[bi]
                kc = order[j]
                n_kc = len(order)
                ht, hp, q0 = h // 2, (h % 2) * 64, qj * CQ
                if j == 0:
                    pv_tiles[bi] = psum.tile([D + 1, CQ], f32, tag="pv",
                                             bufs=2, name="pv")
                pv = pv_tiles[bi]
                pt, qoff, width = pts.pop((bi, kc))
                nc.tensor.matmul(
                    pv[:, qoff:qoff + width],
                    v_sb[kc][:, h * (D + 1):(h + 1) * (D + 1)],
                    pt[:, 0:width],
                    start=(j == 0), stop=(j == n_kc - 1))
                if j != n_kc - 1:
                    continue
                # block complete: evacuate PV to SBUF (frees the PSUM slot
                # early), approx-recip the ones-row, broadcast, multiply.
                del pv_tiles[bi]
                pvs = wpool.tile([D, CQ], f32, tag="pvs", bufs=4)
                nc.vector.tensor_copy(pvs[:], pv[0:D, :])
                ls = wpool.tile([1, CQ], f32, tag="ls", bufs=2)
                nc.vector.tensor_copy(ls[:], pv[D:D + 1, :])
                r = wpool.tile([1, CQ], f32, tag="r", bufs=2)
                with nc.allow_low_precision(reason="softmax denom"):
                    # approx_fast needs SBUF input at partition base 0
                    nc.vector.reciprocal_approx_fast(r[:], ls[:])
                rbs = wpool.tile([D, CQ], f32, tag="rbs", bufs=2)
                nc.gpsimd.partition_broadcast(rbs[:], r[:])
                nc.vector.tensor_mul(
                    ao_sb[ht][hp:hp + D, q0:q0 + CQ],
                    pvs[:], rbs[:])
                if h != H_LOC - 1:
                    continue
                # all heads of this q chunk done: proj + store its token
                # tiles (overlaps the next chunk's attention)
                for tt in range(qj * (CQ // 128), (qj + 1) * (CQ // 128)):
                    for n in range(2):
                        ps = psum.tile([128, CQ], f32, tag="prj", bufs=2,
                                       name="psp")
                        for kc2 in range(2):
                            nc.tensor.matmul(
                                ps[:],
                                ao_sb[kc2][:, tt * 128:(tt + 1) * 128],
                                wp_t[kc2][:, n * CQ:(n + 1) * CQ],
                                start=(kc2 == 0), stop=(kc2 == 1))
                        yt = wpool.tile([128, CQ], f32, tag="y", bufs=4)
                        nc.vector.tensor_copy(yt[:], ps[:])
                        nc.sync.dma_start(
                            Y[tt * 128:(tt + 1) * 128, n * CQ:(n + 1) * CQ],
                            yt[:])

    nc.compile()
    return nc


def _get_compiled():
    global _COMPILED
    if _COMPILED is None:
        _COMPILED = _build()
    return _COMPILED


def make_in_maps(x, Wq, bq, Wk, Wv, Wp):
    in_maps = []
    for c in range(N_CORES):
        b, g = divmod(c, 4)
        sl = slice(g * OL, (g + 1) * OL)
        in_maps.append({
            "A": np.ascontiguousarray(
                x[b].T.reshape(KC, 128, 2, CW).transpose(0, 2, 1, 3)
                .reshape(2 * C, CW)).astype(NP_DT),
            "Wqkv": np.concatenate(
                [Wq[sl].T, Wk[sl].T, Wv[sl].T], axis=1).astype(NP_DT),
            "WpT": np.ascontiguousarray(Wp[:, sl].T).astype(NP_DT),
            "BQ": bq[sl].reshape(OL, 1).astype(np.float32),
        })
    return in_maps


_RUNNER = None


def _make_runner():
    """Build the 8-core shard_map executable once (run_bass_via_pjrt re-jits
    on every call; this caches the traced/compiled callable)."""
    import jax
    from jax.sharding import Mesh, PartitionSpec
    from jax.experimental.shard_map import shard_map
    import concourse.mybir as mybir_
    from concourse import bass2jax

    nc = _get_compiled()
    bass2jax.install_neuronx_cc_hook()

    partition_name = (nc.partition_id_tensor.name
                      if nc.partition_id_tensor else None)
    in_names, out_names, out_avals, zero_outs = [], [], [], []
    for alloc in nc.m.functions[0].allocations:
        if not isinstance(alloc, mybir_.MemoryLocationSet):
            continue
        name = alloc.memorylocations[0].name
        if alloc.kind == "ExternalInput":
            if name != partition_name:
                in_names.append(name)
        elif alloc.kind == "ExternalOutput":
            shape = tuple(alloc.tensor_shape)
            dtype = mybir_.dt.np(alloc.dtype)
            out_names.append(name)
            out_avals.append(jax.core.ShapedArray(shape, dtype))
            zero_outs.append(np.zeros(shape, dtype))
    n_params = len(in_names)
    n_outs = len(out_avals)
    all_in_names = list(in_names) + list(out_names)
    if partition_name is not None:
        all_in_names.append(partition_name)
    donate = tuple(range(n_params, n_params + n_outs))

    def _body(*args):
        operands = list(args)
        if partition_name is not None:
            operands.append(bass2jax.partition_id_tensor())
        outs = bass2jax._bass_exec_p.bind(
            *operands,
            out_avals=tuple(out_avals),
            in_names=tuple(all_in_names),
            out_names=tuple(out_names),
            lowering_input_output_aliases=(),
            sim_require_finite=True,
            sim_require_nnan=True,
            nc=nc,
        )
        return tuple(outs)

    devices = jax.devices()[:N_CORES]
    mesh = Mesh(np.asarray(devices), ("core",))
    in_specs = (PartitionSpec("core"),) * (n_params + n_outs)
    out_specs = (PartitionSpec("core"),) * n_outs
    sharded = jax.jit(
        shard_map(_body, mesh=mesh, in_specs=in_specs, out_specs=out_specs,
                  check_rep=False),
        donate_argnums=donate, keep_unused=True)

    def run(in_maps):
        per_core = [[np.asarray(m[name]) for name in in_names]
                    for m in in_maps]
        concat_in = [
            np.concatenate([per_core[c][i] for c in range(N_CORES)], axis=0)
            for i in range(n_params)]
        concat_zeros = [
            np.zeros((N_CORES * z.shape[0], *z.shape[1:]), z.dtype)
            for z in zero_outs]
        out_arrs = sharded(*concat_in, *concat_zeros)
        return [
            {name: np.asarray(out_arrs[i]).reshape(
                N_CORES, *out_avals[i].shape)[c]
             for i, name in enumerate(out_names)}
            for c in range(N_CORES)]

    return run


def _get_runner():
    global _RUNNER
    if _RUNNER is None:
        _RUNNER = _make_runner()
    return _RUNNER


def _axon_reset():
    try:
        import ctypes
        lib = ctypes.CDLL("/opt/axon/libaxon_pjrt.so")
        if hasattr(lib, "axon_reset"):
            lib.axon_reset()
    except Exception:
        pass


def kernel(x, Wq, bq, Wk, bk, Wv, bv, Wp, bp):
    x = np.asarray(x, dtype=np.float32)
    Wq = np.asarray(Wq, dtype=np.float32)
    bq = np.asarray(bq, dtype=np.float32)
    Wk = np.asarray(Wk, dtype=np.float32)
    Wv = np.asarray(Wv, dtype=np.float32)
    Wp = np.asarray(Wp, dtype=np.float32)
    bv = np.asarray(bv, dtype=np.float32)
    bp = np.asarray(bp, dtype=np.float32)

    in_maps = make_in_maps(x, Wq, bq, Wk, Wv, Wp)

    results = None
    for attempt in range(3):
        try:
            results = _get_runner()(in_maps)
            break
        except Exception:
            if attempt == 2:
                raise
            _axon_reset()  # recover a wedged accelerator and retry

    extra = bv @ Wp.T + bp  # bv/bp fold out of the device kernel
    out = np.empty((B, T, C), dtype=np.float32)
    for b in range(B):
        acc = results[4 * b]["Y"].astype(np.float32)
        for g in range(1, 4):
            acc = acc + results[4 * b + g]["Y"]
        out[b] = acc + extra
    return out



# revision 11
# speedup vs baseline: 1.1016x; 1.1016x over previous
"""Multi-head causal attention (B=2, T=2048, C=1024, H=16, D=64) on 8 trn2 cores.

Sharding: core c -> batch b = c//4, head group g = c%4 (4 heads each),
Megatron-style: QKV column-parallel, proj row-parallel. Partial outputs are
summed on the host; bk is softmax-invariant and dropped, bv/bp fold into a
host-side constant. All matmul operands are bf16 (host-cast), accumulation
and softmax stay fp32.

Device kernel (per core):
  A = x[b].T                       [1024, 2048]  host-transposed + repacked
  Q^T (+bq) / K^T = W.T @ A        [256, 2048]   channels on partitions
  V = A.T @ Wv_loc.T               [2048, 4*(64+1)]  natural layout, a ones
                                   column per head for softmax denominators
  Attention runs on HEAD PAIRS (heads 2hp, 2hp+1 live on partition halves
  0:64 / 64:128 of the qt/kt tiles):
     S^T even/odd = K_h^T.T @ Q_h^T   two K=64 matmuls into adjacent PSUM
                                      banks; disjoint PE row-groups (0,0) /
                                      (64,0) -> they run CONCURRENTLY
     P^T pair = exp(0.125*S^T)        ONE ACT call over both banks (N=2*w)
                                      amortizing the ~352-cycle ACT overhead
     diagonal chunks masked on GpSimd via one paired affine_select
     PV~[65, 2*512] += V~_h.T @ P^T   row 64 accumulates the denominator l
     out^T = PV[0:64] * bcast(1/l)    approx-recip + paired broadcast
  The QKV/V projection chains are INTERLEAVED into the attention stream per
  512-token group, so the PE never idles (and its HAM clock stays warm)
  while the scalar engine grinds exps.
  Y = attn-out^T.T @ Wp_loc.T      [2048, 1024]  bf16 partial, proj
                                   interleaved per q-chunk, summed on host
"""

import sys

sys.path.insert(0, "/opt/trn_rl_repo")

import numpy as np
import ml_dtypes

NP_DT = ml_dtypes.bfloat16

import concourse.bass as bass  # noqa: F401
import concourse.mybir as mybir
import concourse.tile as tile
from concourse import bacc
from concourse.bass_utils import run_bass_kernel_spmd

N_CORES = 8
B, T, C = 2, 2048, 1024
H, D = 16, 64
H_LOC = 4              # heads per core
OL = H_LOC * D         # local channels = 256
CQ = 512               # PSUM-bank q chunk
CK = 128               # k chunk (partition dim)
NT = T // 128          # 16
KC = C // 128          # 8 contraction chunks for QKV

f32 = mybir.dt.float32
bf16 = mybir.dt.bfloat16
DT = bf16  # matmul operand dtype

_COMPILED = None


def _build():
    nc = bacc.Bacc("TRN2", debug=False, num_devices=N_CORES)

    A = nc.dram_tensor("A", [2 * C, T // 2], DT, kind="ExternalInput").ap()
    Wqkv = nc.dram_tensor("Wqkv", [C, 3 * OL], DT, kind="ExternalInput").ap()
    WpT = nc.dram_tensor("WpT", [OL, C], DT, kind="ExternalInput").ap()
    BQ = nc.dram_tensor("BQ", [OL, 1], f32, kind="ExternalInput").ap()
    Y = nc.dram_tensor("Y", [T, C], DT, kind="ExternalOutput").ap()

    Exp = mybir.ActivationFunctionType.Exp

    with tile.TileContext(nc) as tc:
        with tc.tile_pool(name="sbuf", bufs=1) as pool, \
             tc.tile_pool(name="work", bufs=1) as wpool, \
             tc.tile_pool(name="psum", bufs=1, space="PSUM") as psum:

            # ---- resident inputs, per-consumer-slice tiles so subtile DMA
            # deps never overserialize. Load order is the consumption order:
            # bq, (w m0 + a n0) interleaved, w m1..3, wv, a n1, a n2, a n3, wp
            a_t = [[pool.tile([128, CQ], DT, tag=f"A{kc}_{n}", name=f"a{kc}_{n}")
                    for n in range(4)] for kc in range(KC)]
            w_t = [[pool.tile([128, 128], DT, tag=f"W{kc}_{m}", name=f"w{kc}_{m}")
                    for m in range(4)] for kc in range(KC)]
            wv_t = [pool.tile([128, OL], DT, tag=f"WV{kc}",
                              name=f"wv{kc}") for kc in range(KC)]
            bq_t = []
            for m in range(2):
                bq = pool.tile([128, 1], f32, tag=f"BQ{m}", name=f"bq{m}")
                nc.sync.dma_start(bq[:], BQ[m * 128:(m + 1) * 128, :])
                bq_t.append(bq)

            def a_src(kc, n):
                r0 = (kc * 2 + n // 2) * 128
                c0 = (n % 2) * CQ
                return A[r0:r0 + 128, c0:c0 + CQ]

            for kc in range(KC):
                nc.sync.dma_start(w_t[kc][0][:],
                                  Wqkv[kc * 128:(kc + 1) * 128, 0:128])
                nc.sync.dma_start(a_t[kc][0][:], a_src(kc, 0))
            for m in range(1, 4):
                for kc in range(KC):
                    nc.sync.dma_start(
                        w_t[kc][m][:],
                        Wqkv[kc * 128:(kc + 1) * 128, m * 128:(m + 1) * 128])
            for kc in range(KC):
                nc.sync.dma_start(wv_t[kc][:],
                                  Wqkv[kc * 128:(kc + 1) * 128, 2 * OL:3 * OL])
            for n in range(1, 4):
                for kc in range(KC):
                    nc.sync.dma_start(a_t[kc][n][:], a_src(kc, n))
            wp_t = []
            for kc in range(2):
                wp = pool.tile([128, C], DT, tag=f"WP{kc}", name=f"wp{kc}")
                nc.sync.dma_start(wp[:], WpT[kc * 128:(kc + 1) * 128, :])
                wp_t.append(wp)

            # ---- persistent intermediates (qt/kt per 512-col chunk so the
            # interleaved attention never waits on unrelated chunk writes)
            qt_sb = [[pool.tile([128, CQ], DT, tag=f"QT{i}_{n}",
                                name=f"qt{i}_{n}") for n in range(4)]
                     for i in range(2)]
            kt_sb = [[pool.tile([128, CQ], DT, tag=f"KT{i}_{n}",
                                name=f"kt{i}_{n}") for n in range(4)]
                     for i in range(2)]
            v_sb = [pool.tile([128, H_LOC * (D + 1)], DT, tag=f"V{i}",
                              name=f"v{i}") for i in range(NT)]
            ao_sb = [pool.tile([128, T], DT, tag=f"AO{i}", name=f"ao{i}")
                     for i in range(2)]

            # ones columns for the softmax denominators: one strided memset
            # per V tile, done up front off the critical path
            for tt in range(NT):
                nc.vector.memset(
                    v_sb[tt].rearrange("p (h x) -> p h x", x=D + 1)[:, :, D:D + 1],
                    1.0)
            # warm the GpSimd ucode paths and preload the ACT exp table so
            # neither cold-start lands mid-attention
            warm = wpool.tile([128, 8], f32, tag="warm")
            nc.vector.memset(warm[:], 1.0)
            nc.gpsimd.affine_select(
                out=warm[:], in_=warm[:],
                compare_op=mybir.AluOpType.is_ge, fill=0.0, base=0,
                pattern=[[1, 8]], channel_multiplier=-1)
            warm2 = wpool.tile([128, 8], f32, tag="warm2")
            nc.gpsimd.partition_broadcast(warm2[:], warm[0:1, :])
            warm3 = wpool.tile([128, 8], f32, tag="warm3")
            nc.scalar.activation(warm3[:], warm2[:], Exp, scale=0.001)

            # ---- QKV / V projection chain emitters (one chain each) ----
            def emit_qk_chain(n, m):
                ps = psum.tile([128, CQ], f32, tag="prj", bufs=2, name="ps")
                for kc in range(KC):
                    nc.tensor.matmul(
                        ps[:], w_t[kc][m][:], a_t[kc][n][:],
                        start=(kc == 0), stop=(kc == KC - 1))
                if m < 2:
                    nc.vector.tensor_scalar_add(
                        qt_sb[m][n][:], ps[:], bq_t[m][:, 0:1])
                else:
                    nc.vector.tensor_copy(kt_sb[m - 2][n][:], ps[:])

            def emit_v_chain(tt):
                ps = psum.tile([128, CQ], f32, tag="prj", bufs=2,
                               name="psv")[:, 0:OL]
                for kc in range(KC):
                    nc.tensor.matmul(
                        ps[:],
                        a_t[kc][tt // 4][:, (tt % 4) * 128:(tt % 4 + 1) * 128],
                        wv_t[kc][:],
                        start=(kc == 0), stop=(kc == KC - 1))
                nc.vector.tensor_copy(
                    v_sb[tt].rearrange("p (h x) -> p h x", x=D + 1)[:, :, 0:D],
                    ps.rearrange("p (h x) -> p h x", x=D))

            def emit_proj(qj):
                for tt in range(qj * 4, (qj + 1) * 4):
                    for n2 in range(2):
                        ps = psum.tile([128, CQ], f32, tag="prj", bufs=2,
                                       name="psp")
                        for kc2 in range(2):
                            nc.tensor.matmul(
                                ps[:],
                                ao_sb[kc2][:, tt * 128:(tt + 1) * 128],
                                wp_t[kc2][:, n2 * CQ:(n2 + 1) * CQ],
                                start=(kc2 == 0), stop=(kc2 == 1))
                        yt = wpool.tile([128, CQ], DT, tag="y", bufs=4)
                        nc.vector.tensor_copy(yt[:], ps[:])
                        nc.sync.dma_start(
                            Y[tt * 128:(tt + 1) * 128, n2 * CQ:(n2 + 1) * CQ],
                            yt[:])

            # ---- head-pair flash attention, software-pipelined ----
            blocks = []
            group_start = []
            fi = 0
            for qj in range(4):
                group_start.append(fi)
                for hp in range(2):
                    n_kc = (qj + 1) * 4
                    order = list(range(qj * 4, n_kc)) + list(range(0, qj * 4))
                    blocks.append((qj, hp, order))
                    fi += len(order)
            flat = [(bi, j) for bi, (_, _, order) in enumerate(blocks)
                    for j in range(len(order))]
            # Later groups' QKV/V chains are spread one-at-a-time through
            # the earlier groups' attention stream: the PE absorbs them in
            # its ACT-bound slack, and they keep its HAM clock warm. Each
            # group g's chains must all be emitted before the S-emitter
            # (which runs LOOKAHEAD ahead) reaches group_start[g].
            inject = {
                0: [(emit_qk_chain, (1, 0)), (emit_qk_chain, (1, 1))],
                1: [(emit_qk_chain, (1, 2)), (emit_qk_chain, (1, 3))],
                2: [(emit_v_chain, (4,)), (emit_v_chain, (5,))],
                3: [(emit_v_chain, (6,)), (emit_v_chain, (7,))],
                6: [(emit_qk_chain, (2, 0))],
                8: [(emit_qk_chain, (2, 1))],
                10: [(emit_qk_chain, (2, 2))],
                12: [(emit_qk_chain, (2, 3))],
                14: [(emit_v_chain, (8,))],
                16: [(emit_v_chain, (9,))],
                18: [(emit_v_chain, (10,))],
                20: [(emit_v_chain, (11,))],
                24: [(emit_qk_chain, (3, 0))],
                27: [(emit_qk_chain, (3, 1))],
                30: [(emit_qk_chain, (3, 2))],
                33: [(emit_qk_chain, (3, 3))],
                36: [(emit_v_chain, (12,))],
                39: [(emit_v_chain, (13,))],
                42: [(emit_v_chain, (14,))],
                45: [(emit_v_chain, (15,))],
            }
            LOOKAHEAD = 3
            pv_tiles = {}
            pts = {}

            # group 0's QKV / V chains must precede the pipeline seed
            for m in range(4):
                emit_qk_chain(0, m)
            for tt in range(4):
                emit_v_chain(tt)

            def emit_s(idx):
                bi, j = flat[idx]
                qj, hp, order = blocks[bi]
                kc = order[j]
                q0 = qj * CQ
                qoff = max(0, kc * CK - q0)   # causal trim: 0/128/256/384
                width = CQ - qoff
                sp = psum.tile([128, 2 * CQ], f32, tag="sp", bufs=2,
                               name="sp")
                # the two heads use disjoint PE row groups -> concurrent
                nc.tensor.matmul(
                    sp[:, 0:width],
                    kt_sb[hp][kc // 4][0:D, (kc % 4) * 128:(kc % 4 + 1) * 128],
                    qt_sb[hp][qj][0:D, qoff:CQ],
                    start=True, stop=True)
                nc.tensor.matmul(
                    sp[:, CQ:CQ + width],
                    kt_sb[hp][kc // 4][D:128, (kc % 4) * 128:(kc % 4 + 1) * 128],
                    qt_sb[hp][qj][D:128, qoff:CQ],
                    start=True, stop=True)
                pt = wpool.tile([128, 2 * CQ], DT, tag="pT", bufs=6)
                sview = sp.rearrange("p (two q) -> p two q", two=2)[:, :, 0:width]
                pview = pt.rearrange("p (two q) -> p two q", two=2)[:, :, 0:width]
                nc.scalar.activation(pview, sview, Exp, scale=1.0 / 8.0)
                if kc >= qj * 4:   # diagonal chunk: mask q < k on both heads
                    nc.gpsimd.affine_select(
                        out=pview, in_=pview,
                        compare_op=mybir.AluOpType.is_ge,
                        fill=0.0, base=0,
                        pattern=[[0, 2], [1, width]], channel_multiplier=-1)
                pts[(bi, kc)] = (pt, qoff, width)

            for idx in range(min(LOOKAHEAD, len(flat))):
                emit_s(idx)
            for i, (bi, j) in enumerate(flat):
                qj, hp, order = blocks[bi]
                kc = order[j]
                n_kc = len(order)
                q0 = qj * CQ
                if j == 0:
                    pv_tiles[bi] = psum.tile([D + 1, 2 * CQ], f32, tag="pv",
                                             bufs=1, name="pv")
                pv = pv_tiles[bi]
                pt, qoff, width = pts.pop((bi, kc))
                nc.tensor.matmul(
                    pv[:, qoff:qoff + width],
                    v_sb[kc][:, (2 * hp) * (D + 1):(2 * hp + 1) * (D + 1)],
                    pt[:, 0:width],
                    start=(j == 0), stop=(j == n_kc - 1))
                nc.tensor.matmul(
                    pv[:, CQ + qoff:CQ + qoff + width],
                    v_sb[kc][:, (2 * hp + 1) * (D + 1):(2 * hp + 2) * (D + 1)],
                    pt[:, CQ:CQ + width],
                    start=(j == 0), stop=(j == n_kc - 1))
                for fn, args in inject.get(i, ()):
                    fn(*args)
                if i + LOOKAHEAD < len(flat):
                    emit_s(i + LOOKAHEAD)
                if j != n_kc - 1:
                    continue
                # block complete: evacuate both heads' PV, recip the paired
                # ones-row, broadcast, and scale into ao.
                del pv_tiles[bi]
                pvs = wpool.tile([D, 2 * CQ], f32, tag="pvs", bufs=2)
                nc.vector.tensor_copy(pvs[:], pv[0:D, :])
                ls = wpool.tile([1, 2 * CQ], f32, tag="ls", bufs=2)
                nc.vector.tensor_copy(ls[:], pv[D:D + 1, :])
                r = wpool.tile([1, 2 * CQ], f32, tag="r", bufs=2)
                with nc.allow_low_precision(reason="softmax denom"):
                    # approx_fast needs SBUF input at partition base 0
                    nc.vector.reciprocal_approx_fast(r[:], ls[:])
                rbs = wpool.tile([D, 2 * CQ], f32, tag="rbs", bufs=2)
                nc.gpsimd.partition_broadcast(rbs[:], r[:])
                nc.vector.tensor_mul(
                    ao_sb[hp][0:D, q0:q0 + CQ], pvs[:, 0:CQ], rbs[:, 0:CQ])
                nc.vector.tensor_mul(
                    ao_sb[hp][D:128, q0:q0 + CQ], pvs[:, CQ:2 * CQ],
                    rbs[:, CQ:2 * CQ])
                if hp == 1:
                    # both head pairs of this q chunk done: proj + store
                    emit_proj(qj)

    nc.compile()
    return nc


def _get_compiled():
    global _COMPILED
    if _COMPILED is None:
        _COMPILED = _build()
    return _COMPILED


def make_in_maps(x, Wq, bq, Wk, Wv, Wp):
    in_maps = []
    for c in range(N_CORES):
        b, g = divmod(c, 4)
        sl = slice(g * OL, (g + 1) * OL)
        in_maps.append({
            "A": np.ascontiguousarray(
                x[b].T.reshape(KC, 128, 2, T // 2).transpose(0, 2, 1, 3)
                .reshape(2 * C, T // 2)).astype(NP_DT).reshape(2 * C, T // 2),
            "Wqkv": np.concatenate(
                [Wq[sl].T, Wk[sl].T, Wv[sl].T], axis=1).astype(NP_DT),
            "WpT": np.ascontiguousarray(Wp[:, sl].T).astype(NP_DT),
            "BQ": bq[sl].reshape(OL, 1).astype(np.float32),
        })
    return in_maps


_RUNNER = None


def _make_runner():
    """Build the 8-core shard_map executable once (run_bass_via_pjrt re-jits
    on every call; this caches the traced/compiled callable)."""
    import jax
    from jax.sharding import Mesh, PartitionSpec
    from jax.experimental.shard_map import shard_map
    import concourse.mybir as mybir_
    from concourse import bass2jax

    nc = _get_compiled()
    bass2jax.install_neuronx_cc_hook()

    partition_name = (nc.partition_id_tensor.name
                      if nc.partition_id_tensor else None)
    in_names, out_names, out_avals, zero_outs = [], [], [], []
    for alloc in nc.m.functions[0].allocations:
        if not isinstance(alloc, mybir_.MemoryLocationSet):
            continue
        name = alloc.memorylocations[0].name
        if alloc.kind == "ExternalInput":
            if name != partition_name:
                in_names.append(name)
        elif alloc.kind == "ExternalOutput":
            shape = tuple(alloc.tensor_shape)
            dtype = mybir_.dt.np(alloc.dtype)
            out_names.append(name)
            out_avals.append(jax.core.ShapedArray(shape, dtype))
            zero_outs.append(np.zeros(shape, dtype))
    n_params = len(in_names)
    n_outs = len(out_avals)
    all_in_names = list(in_names) + list(out_names)
    if partition_name is not None:
        all_in_names.append(partition_name)
    donate = tuple(range(n_params, n_params + n_outs))

    def _body(*args):
        operands = list(args)
        if partition_name is not None:
            operands.append(bass2jax.partition_id_tensor())
        outs = bass2jax._bass_exec_p.bind(
            *operands,
            out_avals=tuple(out_avals),
            in_names=tuple(all_in_names),
            out_names=tuple(out_names),
            lowering_input_output_aliases=(),
            sim_require_finite=True,
            sim_require_nnan=True,
            nc=nc,
        )
        return tuple(outs)

    devices = jax.devices()[:N_CORES]
    mesh = Mesh(np.asarray(devices), ("core",))
    in_specs = (PartitionSpec("core"),) * (n_params + n_outs)
    out_specs = (PartitionSpec("core"),) * n_outs
    sharded = jax.jit(
        shard_map(_body, mesh=mesh, in_specs=in_specs, out_specs=out_specs,
                  check_rep=False),
        donate_argnums=donate, keep_unused=True)

    def run(in_maps):
        per_core = [[np.asarray(m[name]) for name in in_names]
                    for m in in_maps]
        concat_in = [
            np.concatenate([per_core[c][i] for c in range(N_CORES)], axis=0)
            for i in range(n_params)]
        concat_zeros = [
            np.zeros((N_CORES * z.shape[0], *z.shape[1:]), z.dtype)
            for z in zero_outs]
        out_arrs = sharded(*concat_in, *concat_zeros)
        return [
            {name: np.asarray(out_arrs[i]).reshape(
                N_CORES, *out_avals[i].shape)[c]
             for i, name in enumerate(out_names)}
            for c in range(N_CORES)]

    return run


def _get_runner():
    global _RUNNER
    if _RUNNER is None:
        _RUNNER = _make_runner()
    return _RUNNER


def _axon_reset():
    try:
        import ctypes
        lib = ctypes.CDLL("/opt/axon/libaxon_pjrt.so")
        if hasattr(lib, "axon_reset"):
            lib.axon_reset()
    except Exception:
        pass


def kernel(x, Wq, bq, Wk, bk, Wv, bv, Wp, bp):
    x = np.asarray(x, dtype=np.float32)
    Wq = np.asarray(Wq, dtype=np.float32)
    bq = np.asarray(bq, dtype=np.float32)
    Wk = np.asarray(Wk, dtype=np.float32)
    Wv = np.asarray(Wv, dtype=np.float32)
    Wp = np.asarray(Wp, dtype=np.float32)
    bv = np.asarray(bv, dtype=np.float32)
    bp = np.asarray(bp, dtype=np.float32)

    in_maps = make_in_maps(x, Wq, bq, Wk, Wv, Wp)

    results = None
    for attempt in range(3):
        try:
            results = _get_runner()(in_maps)
            break
        except Exception:
            if attempt == 2:
                raise
            _axon_reset()  # recover a wedged accelerator and retry

    extra = bv @ Wp.T + bp  # bv/bp fold out of the device kernel
    out = np.empty((B, T, C), dtype=np.float32)
    for b in range(B):
        acc = results[4 * b]["Y"].astype(np.float32)
        for g in range(1, 4):
            acc = acc + results[4 * b + g]["Y"].astype(np.float32)
        out[b] = acc + extra
    return out


# revision 12
# speedup vs baseline: 1.2888x; 1.1700x over previous
"""Multi-head causal attention (B=2, T=2048, C=1024, H=16, D=64) on 8 trn2 cores.

Sharding: core c -> batch b = c//4, head group g = c%4 (4 heads each),
Megatron-style: QKV column-parallel, proj row-parallel. Partial outputs are
summed on the host; bk is softmax-invariant and dropped, bv/bp fold into a
host-side constant. All matmul operands are bf16 (host-cast), accumulation
and softmax stay fp32.

Device kernel (per core):
  All inputs are host-packed into a handful of big partition-major tiles so
  the whole load is 8 dma_starts (each costs ~680ns serialized on the sync
  engine - per-slice loads would gate the kernel on sync for ~50us).
  Q^T (+bq) / K^T = W.T @ A        [256, 2048]   channels on partitions
  V = A.T @ Wv_loc.T               [2048, 4*(64+1)]  natural layout, a ones
                                   column per head for softmax denominators
  Attention runs on HEAD PAIRS (heads 2hp, 2hp+1 live on partition halves
  0:64 / 64:128 of the qt/kt tiles):
     S^T even/odd = K_h^T.T @ Q_h^T   two K=64 matmuls into adjacent PSUM
                                      banks; disjoint PE row-groups (0,0) /
                                      (64,0) -> they run CONCURRENTLY
     P^T pair = exp(0.125*S^T)        ONE ACT call over both banks (N=2*w)
                                      amortizing the ~352-cycle ACT overhead
     diagonal chunks masked on GpSimd via one paired affine_select
     PV~[65, 2*512] += V~_h.T @ P^T   row 64 accumulates the denominator l
     out^T = PV[0:64] * bcast(1/l)    approx-recip + paired broadcast,
                                      multiplied straight out of PSUM
  The QKV/V projection chains are SPREAD one-at-a-time through the attention
  stream, so the PE never idles (and its HAM clock stays warm) while the
  scalar engine grinds exps.
  Y = attn-out^T.T @ Wp_loc.T      [2048, 1024]  bf16 partial, proj
                                   interleaved per q-chunk, summed on host
"""

import sys

sys.path.insert(0, "/opt/trn_rl_repo")

import numpy as np
import ml_dtypes

NP_DT = ml_dtypes.bfloat16

import concourse.bass as bass  # noqa: F401
import concourse.mybir as mybir
import concourse.tile as tile
from concourse import bacc
from concourse.bass_utils import run_bass_kernel_spmd

N_CORES = 8
B, T, C = 2, 2048, 1024
H, D = 16, 64
H_LOC = 4              # heads per core
OL = H_LOC * D         # local channels = 256
CQ = 512               # PSUM-bank q chunk
CK = 128               # k chunk (partition dim)
NT = T // 128          # 16
KC = C // 128          # 8 contraction chunks for QKV

f32 = mybir.dt.float32
bf16 = mybir.dt.bfloat16
DT = bf16  # matmul operand dtype

_COMPILED = None


def _build():
    nc = bacc.Bacc("TRN2", debug=False, num_devices=N_CORES)

    # host-packed partition-major inputs (see make_in_maps)
    AB_d = nc.dram_tensor("AB", [128, 4 * KC * CQ], DT, kind="ExternalInput").ap()
    WB_d = nc.dram_tensor("WB", [128, KC * CQ], DT, kind="ExternalInput").ap()
    WVB_d = nc.dram_tensor("WVB", [128, KC * OL], DT, kind="ExternalInput").ap()
    WPB_d = nc.dram_tensor("WPB", [128, 2 * C], DT, kind="ExternalInput").ap()
    BQ_d = nc.dram_tensor("BQ", [128, 2], f32, kind="ExternalInput").ap()
    Y = nc.dram_tensor("Y", [T, C], DT, kind="ExternalOutput").ap()

    Exp = mybir.ActivationFunctionType.Exp

    with tile.TileContext(nc) as tc:
        with tc.tile_pool(name="sbuf", bufs=1) as pool, \
             tc.tile_pool(name="work", bufs=1) as wpool, \
             tc.tile_pool(name="psum", bufs=1, space="PSUM") as psum:

            # ---- resident inputs: 8 dma_starts in consumption order ----
            bq2 = pool.tile([128, 2], f32, tag="BQ", name="bq2")
            nc.sync.dma_start(bq2[:], BQ_d[:, :])
            wB = pool.tile([128, KC * CQ], DT, tag="WB", name="wB")
            nc.sync.dma_start(wB[:], WB_d[:, :])
            aB = [pool.tile([128, KC * CQ], DT, tag=f"AB{n}", name=f"aB{n}")
                  for n in range(4)]
            nc.sync.dma_start(aB[0][:], AB_d[:, 0:KC * CQ])
            wvB = pool.tile([128, KC * OL], DT, tag="WVB", name="wvB")
            nc.sync.dma_start(wvB[:], WVB_d[:, :])
            for n in range(1, 4):
                nc.sync.dma_start(aB[n][:],
                                  AB_d[:, n * KC * CQ:(n + 1) * KC * CQ])
            wpB = pool.tile([128, 2 * C], DT, tag="WPB", name="wpB")
            nc.sync.dma_start(wpB[:], WPB_d[:, :])

            # ---- persistent intermediates (qt/kt per 512-col chunk so the
            # interleaved attention never waits on unrelated chunk writes)
            qt_sb = [[pool.tile([128, CQ], DT, tag=f"QT{i}_{n}",
                                name=f"qt{i}_{n}") for n in range(4)]
                     for i in range(2)]
            kt_sb = [[pool.tile([128, CQ], DT, tag=f"KT{i}_{n}",
                                name=f"kt{i}_{n}") for n in range(4)]
                     for i in range(2)]
            v_sb = [pool.tile([128, H_LOC * (D + 1)], DT, tag=f"V{i}",
                              name=f"v{i}") for i in range(NT)]
            ao_sb = [pool.tile([128, T], DT, tag=f"AO{i}", name=f"ao{i}")
                     for i in range(2)]

            # ones columns for the softmax denominators: one strided memset
            # per V tile, done up front off the critical path
            for tt in range(NT):
                nc.vector.memset(
                    v_sb[tt].rearrange("p (h x) -> p h x", x=D + 1)[:, :, D:D + 1],
                    1.0)
            # warm the GpSimd ucode paths and preload the ACT exp table so
            # neither cold-start lands mid-attention
            warm = wpool.tile([128, 8], f32, tag="warm")
            nc.vector.memset(warm[:], 1.0)
            nc.gpsimd.affine_select(
                out=warm[:], in_=warm[:],
                compare_op=mybir.AluOpType.is_ge, fill=0.0, base=0,
                pattern=[[1, 8]], channel_multiplier=-1)
            warm2 = wpool.tile([128, 8], f32, tag="warm2")
            nc.gpsimd.partition_broadcast(warm2[:], warm[0:1, :])
            warm3 = wpool.tile([128, 8], f32, tag="warm3")
            nc.scalar.activation(warm3[:], warm2[:], Exp, scale=0.001)

            # ---- QKV / V projection chain emitters (one chain each) ----
            def emit_qk_chain(n, m):
                ps = psum.tile([128, CQ], f32, tag="prj", bufs=2, name="ps")
                for kc in range(KC):
                    nc.tensor.matmul(
                        ps[:],
                        wB[:, kc * CQ + m * 128:kc * CQ + (m + 1) * 128],
                        aB[n][:, kc * CQ:(kc + 1) * CQ],
                        start=(kc == 0), stop=(kc == KC - 1))
                if m < 2:
                    nc.vector.tensor_scalar_add(
                        qt_sb[m][n][:], ps[:], bq2[:, m:m + 1])
                else:
                    nc.vector.tensor_copy(kt_sb[m - 2][n][:], ps[:])

            def emit_v_chain(tt):
                ps = psum.tile([128, CQ], f32, tag="prj", bufs=2,
                               name="psv")[:, 0:OL]
                for kc in range(KC):
                    nc.tensor.matmul(
                        ps[:],
                        aB[tt // 4][:, kc * CQ + (tt % 4) * 128:
                                    kc * CQ + (tt % 4 + 1) * 128],
                        wvB[:, kc * OL:(kc + 1) * OL],
                        start=(kc == 0), stop=(kc == KC - 1))
                nc.vector.tensor_copy(
                    v_sb[tt].rearrange("p (h x) -> p h x", x=D + 1)[:, :, 0:D],
                    ps.rearrange("p (h x) -> p h x", x=D))

            def emit_proj(qj):
                for tt in range(qj * 4, (qj + 1) * 4):
                    yt = wpool.tile([128, C], DT, tag="y", bufs=4)
                    for n2 in range(2):
                        ps = psum.tile([128, CQ], f32, tag="prj", bufs=2,
                                       name="psp")
                        for kc2 in range(2):
                            nc.tensor.matmul(
                                ps[:],
                                ao_sb[kc2][:, tt * 128:(tt + 1) * 128],
                                wpB[:, kc2 * C + n2 * CQ:kc2 * C + (n2 + 1) * CQ],
                                start=(kc2 == 0), stop=(kc2 == 1))
                        nc.vector.tensor_copy(
                            yt[:, n2 * CQ:(n2 + 1) * CQ], ps[:])
                    nc.sync.dma_start(Y[tt * 128:(tt + 1) * 128, :], yt[:])

            # ---- head-pair flash attention, software-pipelined ----
            blocks = []
            group_start = []
            fi = 0
            for qj in range(4):
                group_start.append(fi)
                for hp in range(2):
                    n_kc = (qj + 1) * 4
                    order = list(range(qj * 4, n_kc)) + list(range(0, qj * 4))
                    blocks.append((qj, hp, order))
                    fi += len(order)
            flat = [(bi, j) for bi, (_, _, order) in enumerate(blocks)
                    for j in range(len(order))]
            # Later groups' QKV/V chains are spread one-at-a-time through
            # the earlier groups' attention stream: the PE absorbs them in
            # its ACT-bound slack and they keep its HAM clock warm. Group
            # g's chains must be emitted before the S-emitter (LOOKAHEAD
            # ahead) reaches group_start[g]; placements avoid the proj
            # bursts at flat indices 7/23/47.
            inject = {
                0: [(emit_qk_chain, (1, 0)), (emit_qk_chain, (1, 1))],
                1: [(emit_qk_chain, (1, 2)), (emit_qk_chain, (1, 3))],
                2: [(emit_v_chain, (4,)), (emit_v_chain, (5,))],
                3: [(emit_v_chain, (6,)), (emit_v_chain, (7,))],
                9: [(emit_qk_chain, (2, 0))],
                11: [(emit_qk_chain, (2, 1))],
                13: [(emit_qk_chain, (2, 2))],
                15: [(emit_qk_chain, (2, 3))],
                17: [(emit_v_chain, (8,))],
                18: [(emit_v_chain, (9,))],
                19: [(emit_v_chain, (10,))],
                21: [(emit_v_chain, (11,))],
                26: [(emit_qk_chain, (3, 0))],
                29: [(emit_qk_chain, (3, 1))],
                32: [(emit_qk_chain, (3, 2))],
                35: [(emit_qk_chain, (3, 3))],
                38: [(emit_v_chain, (12,))],
                41: [(emit_v_chain, (13,))],
                44: [(emit_v_chain, (14,))],
                45: [(emit_v_chain, (15,))],
            }
            LOOKAHEAD = 3
            pv_tiles = {}
            pts = {}

            # group 0's QKV / V chains must precede the pipeline seed
            for m in range(4):
                emit_qk_chain(0, m)
            for tt in range(4):
                emit_v_chain(tt)

            def emit_s(idx):
                bi, j = flat[idx]
                qj, hp, order = blocks[bi]
                kc = order[j]
                q0 = qj * CQ
                qoff = max(0, kc * CK - q0)   # causal trim: 0/128/256/384
                width = CQ - qoff
                sp = psum.tile([128, 2 * CQ], f32, tag="sp", bufs=2,
                               name="sp")
                # the two heads use disjoint PE row groups -> concurrent
                nc.tensor.matmul(
                    sp[:, 0:width],
                    kt_sb[hp][kc // 4][0:D, (kc % 4) * 128:(kc % 4 + 1) * 128],
                    qt_sb[hp][qj][0:D, qoff:CQ],
                    start=True, stop=True)
                nc.tensor.matmul(
                    sp[:, CQ:CQ + width],
                    kt_sb[hp][kc // 4][D:128, (kc % 4) * 128:(kc % 4 + 1) * 128],
                    qt_sb[hp][qj][D:128, qoff:CQ],
                    start=True, stop=True)
                pt = wpool.tile([128, 2 * CQ], DT, tag="pT", bufs=6)
                sview = sp.rearrange("p (two q) -> p two q", two=2)[:, :, 0:width]
                pview = pt.rearrange("p (two q) -> p two q", two=2)[:, :, 0:width]
                nc.scalar.activation(pview, sview, Exp, scale=1.0 / 8.0)
                if kc >= qj * 4:   # diagonal chunk: mask q < k on both heads
                    nc.gpsimd.affine_select(
                        out=pview, in_=pview,
                        compare_op=mybir.AluOpType.is_ge,
                        fill=0.0, base=0,
                        pattern=[[0, 2], [1, width]], channel_multiplier=-1)
                pts[(bi, kc)] = (pt, qoff, width)

            for idx in range(min(LOOKAHEAD, len(flat))):
                emit_s(idx)
            for i, (bi, j) in enumerate(flat):
                qj, hp, order = blocks[bi]
                kc = order[j]
                n_kc = len(order)
                q0 = qj * CQ
                if j == 0:
                    pv_tiles[bi] = psum.tile([D + 1, 2 * CQ], f32, tag="pv",
                                             bufs=1, name="pv")
                pv = pv_tiles[bi]
                pt, qoff, width = pts.pop((bi, kc))
                nc.tensor.matmul(
                    pv[:, qoff:qoff + width],
                    v_sb[kc][:, (2 * hp) * (D + 1):(2 * hp + 1) * (D + 1)],
                    pt[:, 0:width],
                    start=(j == 0), stop=(j == n_kc - 1))
                nc.tensor.matmul(
                    pv[:, CQ + qoff:CQ + qoff + width],
                    v_sb[kc][:, (2 * hp + 1) * (D + 1):(2 * hp + 2) * (D + 1)],
                    pt[:, CQ:CQ + width],
                    start=(j == 0), stop=(j == n_kc - 1))
                for fn, args in inject.get(i, ()):
                    fn(*args)
                if i + LOOKAHEAD < len(flat):
                    emit_s(i + LOOKAHEAD)
                if j != n_kc - 1:
                    continue
                # block complete: recip the paired ones-row, broadcast, and
                # scale both heads straight out of PSUM into ao.
                del pv_tiles[bi]
                ls = wpool.tile([1, 2 * CQ], f32, tag="ls", bufs=2)
                nc.vector.tensor_copy(ls[:], pv[D:D + 1, :])
                r = wpool.tile([1, 2 * CQ], f32, tag="r", bufs=2)
                with nc.allow_low_precision(reason="softmax denom"):
                    # approx_fast needs SBUF input at partition base 0
                    nc.vector.reciprocal_approx_fast(r[:], ls[:])
                rbs = wpool.tile([D, 2 * CQ], f32, tag="rbs", bufs=2)
                nc.gpsimd.partition_broadcast(rbs[:], r[:])
                nc.vector.tensor_mul(
                    ao_sb[hp][0:D, q0:q0 + CQ], pv[0:D, 0:CQ], rbs[:, 0:CQ])
                nc.vector.tensor_mul(
                    ao_sb[hp][D:128, q0:q0 + CQ], pv[0:D, CQ:2 * CQ],
                    rbs[:, CQ:2 * CQ])
                if hp == 1:
                    # both head pairs of this q chunk done: proj + store
                    emit_proj(qj)

    nc.compile()
    return nc


def _get_compiled():
    global _COMPILED
    if _COMPILED is None:
        _COMPILED = _build()
    return _COMPILED


def make_in_maps(x, Wq, bq, Wk, Wv, Wp):
    in_maps = []
    for c in range(N_CORES):
        b, g = divmod(c, 4)
        sl = slice(g * OL, (g + 1) * OL)
        XT = np.ascontiguousarray(x[b].T)                      # [C, T]
        AB = (XT.reshape(KC, 128, 4, CQ).transpose(1, 2, 0, 3)
              .reshape(128, 4 * KC * CQ))
        WQK = np.concatenate([Wq[sl].T, Wk[sl].T], axis=1)     # [C, 512]
        WB = WQK.reshape(KC, 128, CQ).transpose(1, 0, 2).reshape(128, KC * CQ)
        WVB = (Wv[sl].T.reshape(KC, 128, OL).transpose(1, 0, 2)
               .reshape(128, KC * OL))
        WPB = (Wp[:, sl].T.reshape(2, 128, C).transpose(1, 0, 2)
               .reshape(128, 2 * C))
        in_maps.append({
            "AB": np.ascontiguousarray(AB).astype(NP_DT),
            "WB": np.ascontiguousarray(WB).astype(NP_DT),
            "WVB": np.ascontiguousarray(WVB).astype(NP_DT),
            "WPB": np.ascontiguousarray(WPB).astype(NP_DT),
            "BQ": np.ascontiguousarray(bq[sl].reshape(2, 128).T).astype(
                np.float32),
        })
    return in_maps


_RUNNER = None


def _make_runner():
    """Build the 8-core shard_map executable once (run_bass_via_pjrt re-jits
    on every call; this caches the traced/compiled callable)."""
    import jax
    from jax.sharding import Mesh, PartitionSpec
    from jax.experimental.shard_map import shard_map
    import concourse.mybir as mybir_
    from concourse import bass2jax

    nc = _get_compiled()
    bass2jax.install_neuronx_cc_hook()

    partition_name = (nc.partition_id_tensor.name
                      if nc.partition_id_tensor else None)
    in_names, out_names, out_avals, zero_outs = [], [], [], []
    for alloc in nc.m.functions[0].allocations:
        if not isinstance(alloc, mybir_.MemoryLocationSet):
            continue
        name = alloc.memorylocations[0].name
        if alloc.kind == "ExternalInput":
            if name != partition_name:
                in_names.append(name)
        elif alloc.kind == "ExternalOutput":
            shape = tuple(alloc.tensor_shape)
            dtype = mybir_.dt.np(alloc.dtype)
            out_names.append(name)
            out_avals.append(jax.core.ShapedArray(shape, dtype))
            zero_outs.append(np.zeros(shape, dtype))
    n_params = len(in_names)
    n_outs = len(out_avals)
    all_in_names = list(in_names) + list(out_names)
    if partition_name is not None:
        all_in_names.append(partition_name)
    donate = tuple(range(n_params, n_params + n_outs))

    def _body(*args):
        operands = list(args)
        if partition_name is not None:
            operands.append(bass2jax.partition_id_tensor())
        outs = bass2jax._bass_exec_p.bind(
            *operands,
            out_avals=tuple(out_avals),
            in_names=tuple(all_in_names),
            out_names=tuple(out_names),
            lowering_input_output_aliases=(),
            sim_require_finite=True,
            sim_require_nnan=True,
            nc=nc,
        )
        return tuple(outs)

    devices = jax.devices()[:N_CORES]
    mesh = Mesh(np.asarray(devices), ("core",))
    in_specs = (PartitionSpec("core"),) * (n_params + n_outs)
    out_specs = (PartitionSpec("core"),) * n_outs
    sharded = jax.jit(
        shard_map(_body, mesh=mesh, in_specs=in_specs, out_specs=out_specs,
                  check_rep=False),
        donate_argnums=donate, keep_unused=True)

    def run(in_maps):
        per_core = [[np.asarray(m[name]) for name in in_names]
                    for m in in_maps]
        concat_in = [
            np.concatenate([per_core[c][i] for c in range(N_CORES)], axis=0)
            for i in range(n_params)]
        concat_zeros = [
            np.zeros((N_CORES * z.shape[0], *z.shape[1:]), z.dtype)
            for z in zero_outs]
        out_arrs = sharded(*concat_in, *concat_zeros)
        return [
            {name: np.asarray(out_arrs[i]).reshape(
                N_CORES, *out_avals[i].shape)[c]
             for i, name in enumerate(out_names)}
            for c in range(N_CORES)]

    return run


def _get_runner():
    global _RUNNER
    if _RUNNER is None:
        _RUNNER = _make_runner()
    return _RUNNER


def _axon_reset():
    try:
        import ctypes
        lib = ctypes.CDLL("/opt/axon/libaxon_pjrt.so")
        if hasattr(lib, "axon_reset"):
            lib.axon_reset()
    except Exception:
        pass


def kernel(x, Wq, bq, Wk, bk, Wv, bv, Wp, bp):
    x = np.asarray(x, dtype=np.float32)
    Wq = np.asarray(Wq, dtype=np.float32)
    bq = np.asarray(bq, dtype=np.float32)
    Wk = np.asarray(Wk, dtype=np.float32)
    Wv = np.asarray(Wv, dtype=np.float32)
    Wp = np.asarray(Wp, dtype=np.float32)
    bv = np.asarray(bv, dtype=np.float32)
    bp = np.asarray(bp, dtype=np.float32)

    in_maps = make_in_maps(x, Wq, bq, Wk, Wv, Wp)

    results = None
    for attempt in range(3):
        try:
            results = _get_runner()(in_maps)
            break
        except Exception:
            if attempt == 2:
                raise
            _axon_reset()  # recover a wedged accelerator and retry

    extra = bv @ Wp.T + bp  # bv/bp fold out of the device kernel
    out = np.empty((B, T, C), dtype=np.float32)
    for b in range(B):
        acc = results[4 * b]["Y"].astype(np.float32)
        for g in range(1, 4):
            acc = acc + results[4 * b + g]["Y"].astype(np.float32)
        out[b] = acc + extra
    return out


# revision 17
# speedup vs baseline: 1.4267x; 1.1070x over previous
"""Multi-head causal attention (B=2, T=2048, C=1024, H=16, D=64) on 8 trn2 cores.

Sharding: core c -> batch b = c//4, head group g = c%4 (4 heads each),
Megatron-style: QKV column-parallel, proj row-parallel. Partial outputs are
summed on the host; bk is softmax-invariant and dropped, bv/bp fold into a
host-side constant. All matmul operands are bf16 (host-cast), accumulation
and softmax stay fp32.

Device kernel (per core):
  All inputs are host-packed into a handful of big partition-major tiles so
  the whole load is 8 dma_starts (each costs ~680ns serialized on the sync
  engine - per-slice loads would gate the kernel on sync for ~50us).
  Q^T (+bq) / K^T = W.T @ A        [256, 2048]   channels on partitions
  V = A.T @ Wv_loc.T               [2048, 4*(64+1)]  natural layout, a ones
                                   column per head for softmax denominators
  Attention runs on HEAD PAIRS (heads 2hp, 2hp+1 live on partition halves
  0:64 / 64:128 of the qt/kt tiles):
     S^T even/odd = K_h^T.T @ Q_h^T   two K=64 matmuls into adjacent PSUM
                                      banks; disjoint PE row-groups (0,0) /
                                      (64,0) -> they run CONCURRENTLY
     P^T pair = exp(0.125*S^T)        ONE ACT call over both banks (N=2*w)
                                      amortizing the ~352-cycle ACT overhead
     diagonal chunks masked on GpSimd via one paired affine_select
     PV~[65, 2*512] += V~_h.T @ P^T   row 64 accumulates the denominator l
     out^T = PV[0:64] * bcast(1/l)    approx-recip + paired broadcast,
                                      multiplied straight out of PSUM
  The QKV/V projection chains are SPREAD one-at-a-time through the attention
  stream, so the PE never idles (and its HAM clock stays warm) while the
  scalar engine grinds exps.
  Y = attn-out^T.T @ Wp_loc.T      [2048, 1024]  bf16 partial, proj
                                   interleaved per q-chunk, summed on host
"""

import sys

sys.path.insert(0, "/opt/trn_rl_repo")

import numpy as np
import ml_dtypes

NP_DT = ml_dtypes.bfloat16

import concourse.bass as bass  # noqa: F401
import concourse.mybir as mybir
import concourse.tile as tile
from concourse import bacc
from concourse.bass_utils import run_bass_kernel_spmd

N_CORES = 8
B, T, C = 2, 2048, 1024
H, D = 16, 64
H_LOC = 4              # heads per core
OL = H_LOC * D         # local channels = 256
CQ = 512               # PSUM-bank q chunk
CK = 128               # k chunk (partition dim)
NT = T // 128          # 16
KC = C // 128          # 8 contraction chunks for QKV

f32 = mybir.dt.float32
bf16 = mybir.dt.bfloat16
DT = bf16  # matmul operand dtype

_COMPILED = None


def _build():
    nc = bacc.Bacc("TRN2", debug=False, num_devices=N_CORES)

    # host-packed partition-major inputs (see make_in_maps)
    AB_d = nc.dram_tensor("AB", [128, 4 * KC * CQ], DT, kind="ExternalInput").ap()
    WB_d = nc.dram_tensor("WB", [128, KC * CQ], DT, kind="ExternalInput").ap()
    WVB_d = nc.dram_tensor("WVB", [128, KC * OL], DT, kind="ExternalInput").ap()
    WPB_d = nc.dram_tensor("WPB", [128, 2 * C], DT, kind="ExternalInput").ap()
    BQ_d = nc.dram_tensor("BQ", [128, 2], f32, kind="ExternalInput").ap()
    Y = nc.dram_tensor("Y", [T, C], DT, kind="ExternalOutput").ap()

    Exp = mybir.ActivationFunctionType.Exp

    with tile.TileContext(nc) as tc:
        with tc.tile_pool(name="sbuf", bufs=1) as pool, \
             tc.tile_pool(name="work", bufs=1) as wpool, \
             tc.tile_pool(name="psum", bufs=1, space="PSUM") as psum:

            # ---- resident inputs: 8 dma_starts in consumption order ----
            bq2 = pool.tile([128, 2], f32, tag="BQ", name="bq2")
            nc.sync.dma_start(bq2[:], BQ_d[:, :])
            wB = pool.tile([128, KC * CQ], DT, tag="WB", name="wB")
            nc.sync.dma_start(wB[:], WB_d[:, :])
            aB = [pool.tile([128, KC * CQ], DT, tag=f"AB{n}", name=f"aB{n}")
                  for n in range(4)]
            nc.sync.dma_start(aB[0][:], AB_d[:, 0:KC * CQ])
            wvB = pool.tile([128, KC * OL], DT, tag="WVB", name="wvB")
            nc.sync.dma_start(wvB[:], WVB_d[:, :])
            for n in range(1, 4):
                nc.sync.dma_start(aB[n][:],
                                  AB_d[:, n * KC * CQ:(n + 1) * KC * CQ])
            wpB = pool.tile([128, 2 * C], DT, tag="WPB", name="wpB")
            nc.sync.dma_start(wpB[:], WPB_d[:, :])

            # ---- persistent intermediates (qt/kt per 512-col chunk so the
            # interleaved attention never waits on unrelated chunk writes)
            qt_sb = [[pool.tile([128, CQ], DT, tag=f"QT{i}_{n}",
                                name=f"qt{i}_{n}") for n in range(4)]
                     for i in range(2)]
            kt_sb = [[pool.tile([128, CQ], DT, tag=f"KT{i}_{n}",
                                name=f"kt{i}_{n}") for n in range(4)]
                     for i in range(2)]
            v_sb = [pool.tile([128, H_LOC * (D + 1)], DT, tag=f"V{i}",
                              name=f"v{i}") for i in range(NT)]
            ao_sb = [pool.tile([128, T], DT, tag=f"AO{i}", name=f"ao{i}")
                     for i in range(2)]

            # ones columns for the softmax denominators: one strided memset
            # per V tile, done up front off the critical path
            for tt in range(NT):
                nc.vector.memset(
                    v_sb[tt].rearrange("p (h x) -> p h x", x=D + 1)[:, :, D:D + 1],
                    1.0)
            # warm the GpSimd ucode paths and preload the ACT exp table so
            # neither cold-start lands mid-attention
            warm = wpool.tile([128, 8], f32, tag="warm")
            nc.vector.memset(warm[:], 1.0)
            nc.gpsimd.affine_select(
                out=warm[:], in_=warm[:],
                compare_op=mybir.AluOpType.is_ge, fill=0.0, base=0,
                pattern=[[1, 8]], channel_multiplier=-1)
            warm2 = wpool.tile([128, 8], f32, tag="warm2")
            nc.gpsimd.partition_broadcast(warm2[:], warm[0:1, :])
            warm3 = wpool.tile([128, 8], f32, tag="warm3")
            nc.scalar.activation(warm3[:], warm2[:], Exp, scale=0.001)
            # dummy matmuls during the ~15us input-DMA window: ~5us of PE
            # activity flips the HAM clock gate to 8/8 so the first QKV
            # chains run at 2.4GHz instead of 1.2
            wmm = wpool.tile([128, 128], DT, tag="wmm")
            nc.vector.memset(wmm[:], 0.0)
            for _ in range(5):
                pw = psum.tile([128, CQ], f32, tag="prj", bufs=2, name="pw")
                for k in range(8):
                    nc.tensor.matmul(pw[:, 0:128], wmm[:], wmm[:],
                                     start=(k == 0), stop=(k == 7))

            # ---- QKV / V projection chain emitters (one chain each) ----
            def emit_qk_chain(n, m):
                ps = psum.tile([128, CQ], f32, tag="prj", bufs=2, name="ps")
                for kc in range(KC):
                    nc.tensor.matmul(
                        ps[:],
                        wB[:, kc * CQ + m * 128:kc * CQ + (m + 1) * 128],
                        aB[n][:, kc * CQ:(kc + 1) * CQ],
                        start=(kc == 0), stop=(kc == KC - 1))
                if m < 2:
                    nc.vector.tensor_scalar_add(
                        qt_sb[m][n][:], ps[:], bq2[:, m:m + 1])
                else:
                    nc.vector.tensor_copy(kt_sb[m - 2][n][:], ps[:])

            def emit_v_chain(tt):
                ps = psum.tile([128, CQ], f32, tag="prj", bufs=2,
                               name="psv")[:, 0:OL]
                for kc in range(KC):
                    nc.tensor.matmul(
                        ps[:],
                        aB[tt // 4][:, kc * CQ + (tt % 4) * 128:
                                    kc * CQ + (tt % 4 + 1) * 128],
                        wvB[:, kc * OL:(kc + 1) * OL],
                        start=(kc == 0), stop=(kc == KC - 1))
                nc.vector.tensor_copy(
                    v_sb[tt].rearrange("p (h x) -> p h x", x=D + 1)[:, :, 0:D],
                    ps.rearrange("p (h x) -> p h x", x=D))

            yt_tiles = {}

            def emit_proj_chain(tt, n2):
                if n2 == 0:
                    yt_tiles[tt] = wpool.tile([128, C], DT, tag="y", bufs=3,
                                              name="yt")
                yt = yt_tiles[tt]
                ps = psum.tile([128, CQ], f32, tag="prj", bufs=2, name="psp")
                for kc2 in range(2):
                    nc.tensor.matmul(
                        ps[:],
                        ao_sb[kc2][:, tt * 128:(tt + 1) * 128],
                        wpB[:, kc2 * C + n2 * CQ:kc2 * C + (n2 + 1) * CQ],
                        start=(kc2 == 0), stop=(kc2 == 1))
                nc.vector.tensor_copy(yt[:, n2 * CQ:(n2 + 1) * CQ], ps[:])
                if n2 == 1:
                    del yt_tiles[tt]
                    nc.sync.dma_start(Y[tt * 128:(tt + 1) * 128, :], yt[:])

            # ---- head-pair flash attention, software-pipelined ----
            blocks = []
            group_start = []
            fi = 0
            for qj in range(4):
                group_start.append(fi)
                for hp in range(2):
                    n_kc = (qj + 1) * 4
                    order = list(range(qj * 4, n_kc)) + list(range(0, qj * 4))
                    blocks.append((qj, hp, order))
                    fi += len(order)
            flat = [(bi, j) for bi, (_, _, order) in enumerate(blocks)
                    for j in range(len(order))]
            # Later groups' QKV/V chains AND the previous group's proj
            # chains are spread one-at-a-time through the attention stream:
            # the PE absorbs them between S/PV pairs (keeping its HAM clock
            # warm) while the scalar engine grinds exps. QKV chains for
            # group g must be emitted before the S-emitter (LOOKAHEAD
            # ahead) reaches group_start[g].
            inject = {
                0: [(emit_qk_chain, (1, 0)), (emit_qk_chain, (1, 1))],
                1: [(emit_qk_chain, (1, 2)), (emit_qk_chain, (1, 3))],
                2: [(emit_v_chain, (4,)), (emit_v_chain, (5,))],
                3: [(emit_v_chain, (6,)), (emit_v_chain, (7,))],
                # group 1 (flat 8..23) hosts QKV/V(2) + proj(0)
                8: [(emit_qk_chain, (2, 0))],
                9: [(emit_proj_chain, (0, 0))],
                10: [(emit_qk_chain, (2, 1))],
                11: [(emit_proj_chain, (0, 1))],
                12: [(emit_qk_chain, (2, 2))],
                13: [(emit_proj_chain, (1, 0))],
                14: [(emit_qk_chain, (2, 3))],
                15: [(emit_proj_chain, (1, 1))],
                16: [(emit_v_chain, (8,))],
                17: [(emit_proj_chain, (2, 0))],
                18: [(emit_v_chain, (9,))],
                19: [(emit_proj_chain, (2, 1))],
                20: [(emit_v_chain, (10,))],
                21: [(emit_v_chain, (11,))],
                22: [(emit_proj_chain, (3, 0))],
                23: [(emit_proj_chain, (3, 1))],
                # group 2 (flat 24..47) hosts QKV/V(3) + proj(1)
                25: [(emit_proj_chain, (4, 0))],
                26: [(emit_qk_chain, (3, 0))],
                28: [(emit_proj_chain, (4, 1))],
                29: [(emit_qk_chain, (3, 1))],
                31: [(emit_proj_chain, (5, 0))],
                32: [(emit_qk_chain, (3, 2))],
                34: [(emit_proj_chain, (5, 1))],
                35: [(emit_qk_chain, (3, 3))],
                37: [(emit_proj_chain, (6, 0))],
                38: [(emit_v_chain, (12,))],
                39: [(emit_proj_chain, (6, 1))],
                40: [(emit_v_chain, (13,))],
                42: [(emit_v_chain, (14,))],
                43: [(emit_proj_chain, (7, 0))],
                44: [(emit_v_chain, (15,))],
                46: [(emit_proj_chain, (7, 1))],
                # group 3 (flat 48..79) hosts proj(2)
                50: [(emit_proj_chain, (8, 0))],
                53: [(emit_proj_chain, (8, 1))],
                56: [(emit_proj_chain, (9, 0))],
                59: [(emit_proj_chain, (9, 1))],
                62: [(emit_proj_chain, (10, 0))],
                65: [(emit_proj_chain, (10, 1))],
                68: [(emit_proj_chain, (11, 0))],
                71: [(emit_proj_chain, (11, 1))],
            }
            LOOKAHEAD = 3
            pv_tiles = {}
            pts = {}

            # group 0's QKV / V chains must precede the pipeline seed
            for m in range(4):
                emit_qk_chain(0, m)
            for tt in range(4):
                emit_v_chain(tt)

            def emit_s(idx):
                bi, j = flat[idx]
                qj, hp, order = blocks[bi]
                kc = order[j]
                q0 = qj * CQ
                qoff = max(0, kc * CK - q0)   # causal trim: 0/128/256/384
                width = CQ - qoff
                sp = psum.tile([128, 2 * CQ], f32, tag="sp", bufs=2,
                               name="sp")
                # the two heads use disjoint PE row groups -> concurrent
                nc.tensor.matmul(
                    sp[:, 0:width],
                    kt_sb[hp][kc // 4][0:D, (kc % 4) * 128:(kc % 4 + 1) * 128],
                    qt_sb[hp][qj][0:D, qoff:CQ],
                    start=True, stop=True)
                nc.tensor.matmul(
                    sp[:, CQ:CQ + width],
                    kt_sb[hp][kc // 4][D:128, (kc % 4) * 128:(kc % 4 + 1) * 128],
                    qt_sb[hp][qj][D:128, qoff:CQ],
                    start=True, stop=True)
                pt = wpool.tile([128, 2 * CQ], DT, tag="pT", bufs=6)
                sview = sp.rearrange("p (two q) -> p two q", two=2)[:, :, 0:width]
                pview = pt.rearrange("p (two q) -> p two q", two=2)[:, :, 0:width]
                nc.scalar.activation(pview, sview, Exp, scale=1.0 / 8.0)
                if kc >= qj * 4:   # diagonal chunk: mask q < k on both heads
                    nc.gpsimd.affine_select(
                        out=pview, in_=pview,
                        compare_op=mybir.AluOpType.is_ge,
                        fill=0.0, base=0,
                        pattern=[[0, 2], [1, width]], channel_multiplier=-1)
                pts[(bi, kc)] = (pt, qoff, width)

            for idx in range(min(LOOKAHEAD, len(flat))):
                emit_s(idx)
            for i, (bi, j) in enumerate(flat):
                qj, hp, order = blocks[bi]
                kc = order[j]
                n_kc = len(order)
                q0 = qj * CQ
                if j == 0:
                    pv_tiles[bi] = psum.tile([D + 1, 2 * CQ], f32, tag="pv",
                                             bufs=1, name="pv")
                pv = pv_tiles[bi]
                pt, qoff, width = pts.pop((bi, kc))
                nc.tensor.matmul(
                    pv[:, qoff:qoff + width],
                    v_sb[kc][:, (2 * hp) * (D + 1):(2 * hp + 1) * (D + 1)],
                    pt[:, 0:width],
                    start=(j == 0), stop=(j == n_kc - 1))
                nc.tensor.matmul(
                    pv[:, CQ + qoff:CQ + qoff + width],
                    v_sb[kc][:, (2 * hp + 1) * (D + 1):(2 * hp + 2) * (D + 1)],
                    pt[:, CQ:CQ + width],
                    start=(j == 0), stop=(j == n_kc - 1))
                for fn, args in inject.get(i, ()):
                    fn(*args)
                if i + LOOKAHEAD < len(flat):
                    emit_s(i + LOOKAHEAD)
                if j != n_kc - 1:
                    continue
                # block complete: evacuate PV fast (pvs on DVE, the ones-row
                # on the idle-ish scalar engine) so the single pv PSUM slot
                # unlocks for the next block, then recip/broadcast/scale.
                del pv_tiles[bi]
                pvs = wpool.tile([D, 2 * CQ], f32, tag="pvs", bufs=2)
                nc.vector.tensor_copy(pvs[:], pv[0:D, :])
                ls = wpool.tile([1, 2 * CQ], f32, tag="ls", bufs=2)
                nc.scalar.copy(ls[:], pv[D:D + 1, :])
                r = wpool.tile([1, 2 * CQ], f32, tag="r", bufs=2)
                with nc.allow_low_precision(reason="softmax denom"):
                    # approx_fast needs SBUF input at partition base 0
                    nc.vector.reciprocal_approx_fast(r[:], ls[:])
                rbs = wpool.tile([D, 2 * CQ], f32, tag="rbs", bufs=2)
                nc.gpsimd.partition_broadcast(rbs[:], r[:])
                nc.vector.tensor_mul(
                    ao_sb[hp][0:D, q0:q0 + CQ], pvs[:, 0:CQ], rbs[:, 0:CQ])
                nc.vector.tensor_mul(
                    ao_sb[hp][D:128, q0:q0 + CQ], pvs[:, CQ:2 * CQ],
                    rbs[:, CQ:2 * CQ])
                if hp == 1 and qj == 3:
                    # final q chunk: proj burst + store (earlier qj's proj
                    # chains were spread through the stream via inject)
                    for tt in range(12, 16):
                        emit_proj_chain(tt, 0)
                        emit_proj_chain(tt, 1)

    nc.compile()
    return nc


def _get_compiled():
    global _COMPILED
    if _COMPILED is None:
        _COMPILED = _build()
    return _COMPILED


def make_in_maps(x, Wq, bq, Wk, Wv, Wp):
    in_maps = []
    for c in range(N_CORES):
        b, g = divmod(c, 4)
        sl = slice(g * OL, (g + 1) * OL)
        XT = np.ascontiguousarray(x[b].T)                      # [C, T]
        AB = (XT.reshape(KC, 128, 4, CQ).transpose(1, 2, 0, 3)
              .reshape(128, 4 * KC * CQ))
        WQK = np.concatenate([Wq[sl].T, Wk[sl].T], axis=1)     # [C, 512]
        WB = WQK.reshape(KC, 128, CQ).transpose(1, 0, 2).reshape(128, KC * CQ)
        WVB = (Wv[sl].T.reshape(KC, 128, OL).transpose(1, 0, 2)
               .reshape(128, KC * OL))
        WPB = (Wp[:, sl].T.reshape(2, 128, C).transpose(1, 0, 2)
               .reshape(128, 2 * C))
        in_maps.append({
            "AB": np.ascontiguousarray(AB).astype(NP_DT),
            "WB": np.ascontiguousarray(WB).astype(NP_DT),
            "WVB": np.ascontiguousarray(WVB).astype(NP_DT),
            "WPB": np.ascontiguousarray(WPB).astype(NP_DT),
            "BQ": np.ascontiguousarray(bq[sl].reshape(2, 128).T).astype(
                np.float32),
        })
    return in_maps


_RUNNER = None


def _make_runner():
    """Build the 8-core shard_map executable once (run_bass_via_pjrt re-jits
    on every call; this caches the traced/compiled callable)."""
    import jax
    from jax.sharding import Mesh, PartitionSpec
    from jax.experimental.shard_map import shard_map
    import concourse.mybir as mybir_
    from concourse import bass2jax

    nc = _get_compiled()
    bass2jax.install_neuronx_cc_hook()

    partition_name = (nc.partition_id_tensor.name
                      if nc.partition_id_tensor else None)
    in_names, out_names, out_avals, zero_outs = [], [], [], []
    for alloc in nc.m.functions[0].allocations:
        if not isinstance(alloc, mybir_.MemoryLocationSet):
            continue
        name = alloc.memorylocations[0].name
        if alloc.kind == "ExternalInput":
            if name != partition_name:
                in_names.append(name)
        elif alloc.kind == "ExternalOutput":
            shape = tuple(alloc.tensor_shape)
            dtype = mybir_.dt.np(alloc.dtype)
            out_names.append(name)
            out_avals.append(jax.core.ShapedArray(shape, dtype))
            zero_outs.append(np.zeros(shape, dtype))
    n_params = len(in_names)
    n_outs = len(out_avals)
    all_in_names = list(in_names) + list(out_names)
    if partition_name is not None:
        all_in_names.append(partition_name)
    donate = tuple(range(n_params, n_params + n_outs))

    def _body(*args):
        operands = list(args)
        if partition_name is not None:
            operands.append(bass2jax.partition_id_tensor())
        outs = bass2jax._bass_exec_p.bind(
            *operands,
            out_avals=tuple(out_avals),
            in_names=tuple(all_in_names),
            out_names=tuple(out_names),
            lowering_input_output_aliases=(),
            sim_require_finite=True,
            sim_require_nnan=True,
            nc=nc,
        )
        return tuple(outs)

    devices = jax.devices()[:N_CORES]
    mesh = Mesh(np.asarray(devices), ("core",))
    in_specs = (PartitionSpec("core"),) * (n_params + n_outs)
    out_specs = (PartitionSpec("core"),) * n_outs
    sharded = jax.jit(
        shard_map(_body, mesh=mesh, in_specs=in_specs, out_specs=out_specs,
                  check_rep=False),
        donate_argnums=donate, keep_unused=True)

    def run(in_maps):
        per_core = [[np.asarray(m[name]) for name in in_names]
                    for m in in_maps]
        concat_in = [
            np.concatenate([per_core[c][i] for c in range(N_CORES)], axis=0)
            for i in range(n_params)]
        concat_zeros = [
            np.zeros((N_CORES * z.shape[0], *z.shape[1:]), z.dtype)
            for z in zero_outs]
        out_arrs = sharded(*concat_in, *concat_zeros)
        return [
            {name: np.asarray(out_arrs[i]).reshape(
                N_CORES, *out_avals[i].shape)[c]
             for i, name in enumerate(out_names)}
            for c in range(N_CORES)]

    return run


def _get_runner():
    global _RUNNER
    if _RUNNER is None:
        _RUNNER = _make_runner()
    return _RUNNER


def _axon_reset():
    try:
        import ctypes
        lib = ctypes.CDLL("/opt/axon/libaxon_pjrt.so")
        if hasattr(lib, "axon_reset"):
            lib.axon_reset()
    except Exception:
        pass


def kernel(x, Wq, bq, Wk, bk, Wv, bv, Wp, bp):
    x = np.asarray(x, dtype=np.float32)
    Wq = np.asarray(Wq, dtype=np.float32)
    bq = np.asarray(bq, dtype=np.float32)
    Wk = np.asarray(Wk, dtype=np.float32)
    Wv = np.asarray(Wv, dtype=np.float32)
    Wp = np.asarray(Wp, dtype=np.float32)
    bv = np.asarray(bv, dtype=np.float32)
    bp = np.asarray(bp, dtype=np.float32)

    in_maps = make_in_maps(x, Wq, bq, Wk, Wv, Wp)

    results = None
    for attempt in range(3):
        try:
            results = _get_runner()(in_maps)
            break
        except Exception:
            if attempt == 2:
                raise
            _axon_reset()  # recover a wedged accelerator and retry

    extra = bv @ Wp.T + bp  # bv/bp fold out of the device kernel
    out = np.empty((B, T, C), dtype=np.float32)
    for b in range(B):
        acc = results[4 * b]["Y"].astype(np.float32)
        for g in range(1, 4):
            acc = acc + results[4 * b + g]["Y"].astype(np.float32)
        out[b] = acc + extra
    return out


# revision 23
# speedup vs baseline: 1.4302x; 1.0024x over previous
"""Multi-head causal attention (B=2, T=2048, C=1024, H=16, D=64) on 8 trn2 cores.

Sharding: core c -> batch b = c//4, head group g = c%4 (4 heads each),
Megatron-style: QKV column-parallel, proj row-parallel. Partial outputs are
summed on the host; bk is softmax-invariant and dropped, bv/bp fold into a
host-side constant. All matmul operands are bf16 (host-cast), accumulation
and softmax stay fp32.

Device kernel (per core):
  All inputs are host-packed into a handful of big partition-major tiles so
  the whole load is 8 dma_starts (each costs ~680ns serialized on the sync
  engine - per-slice loads would gate the kernel on sync for ~50us).
  Q^T (+bq) / K^T = W.T @ A        [256, 2048]   channels on partitions
  V = A.T @ Wv_loc.T               [2048, 4*(64+1)]  natural layout, a ones
                                   column per head for softmax denominators
  Attention runs on HEAD PAIRS (heads 2hp, 2hp+1 live on partition halves
  0:64 / 64:128 of the qt/kt tiles):
     S^T even/odd = K_h^T.T @ Q_h^T   two K=64 matmuls into adjacent PSUM
                                      banks; disjoint PE row-groups (0,0) /
                                      (64,0) -> they run CONCURRENTLY
     P^T pair = exp(0.125*S^T)        ONE ACT call over both banks (N=2*w)
                                      amortizing the ~352-cycle ACT overhead
     diagonal chunks masked on GpSimd via one paired affine_select
     PV~[65, 2*512] += V~_h.T @ P^T   row 64 accumulates the denominator l
     out^T = PV[0:64] * bcast(1/l)    approx-recip + paired broadcast,
                                      multiplied straight out of PSUM
  The QKV/V projection chains are SPREAD one-at-a-time through the attention
  stream, so the PE never idles (and its HAM clock stays warm) while the
  scalar engine grinds exps.
  Y = attn-out^T.T @ Wp_loc.T      [2048, 1024]  bf16 partial, proj
                                   interleaved per q-chunk, summed on host
"""

import sys

sys.path.insert(0, "/opt/trn_rl_repo")

import numpy as np
import ml_dtypes

NP_DT = ml_dtypes.bfloat16

import concourse.bass as bass  # noqa: F401
import concourse.mybir as mybir
import concourse.tile as tile
from concourse import bacc
from concourse.bass_utils import run_bass_kernel_spmd

N_CORES = 8
B, T, C = 2, 2048, 1024
H, D = 16, 64
H_LOC = 4              # heads per core
OL = H_LOC * D         # local channels = 256
CQ = 512               # PSUM-bank q chunk
CK = 128               # k chunk (partition dim)
NT = T // 128          # 16
KC = C // 128          # 8 contraction chunks for QKV

f32 = mybir.dt.float32
bf16 = mybir.dt.bfloat16
DT = bf16  # matmul operand dtype

_COMPILED = None


def _build():
    nc = bacc.Bacc("TRN2", debug=False, num_devices=N_CORES)

    # host-packed partition-major inputs (see make_in_maps)
    AB_d = nc.dram_tensor("AB", [128, 4 * KC * CQ], DT, kind="ExternalInput").ap()
    WB_d = nc.dram_tensor("WB", [128, KC * CQ], DT, kind="ExternalInput").ap()
    WVB_d = nc.dram_tensor("WVB", [128, KC * OL], DT, kind="ExternalInput").ap()
    WPB_d = nc.dram_tensor("WPB", [128, 2 * C], DT, kind="ExternalInput").ap()
    BQ_d = nc.dram_tensor("BQ", [128, 2], f32, kind="ExternalInput").ap()
    Y = nc.dram_tensor("Y", [T, C], DT, kind="ExternalOutput").ap()

    Exp = mybir.ActivationFunctionType.Exp

    with tile.TileContext(nc) as tc:
        with tc.tile_pool(name="sbuf", bufs=1) as pool, \
             tc.tile_pool(name="work", bufs=1) as wpool, \
             tc.tile_pool(name="psum", bufs=1, space="PSUM") as psum:

            # ---- resident inputs: 8 dma_starts in consumption order ----
            bq2 = pool.tile([128, 2], f32, tag="BQ", name="bq2")
            nc.sync.dma_start(bq2[:], BQ_d[:, :])
            # wB/aB0 land in kc-halves so the first QK chain's early kc
            # matmuls can start ~4.5us before the full tiles arrive
            HB = KC * CQ // 2
            wB = pool.tile([128, KC * CQ], DT, tag="WB", name="wB")
            aB = [pool.tile([128, KC * CQ], DT, tag=f"AB{n}", name=f"aB{n}")
                  for n in range(4)]
            nc.sync.dma_start(wB[:, 0:HB], WB_d[:, 0:HB])
            nc.sync.dma_start(aB[0][:, 0:HB], AB_d[:, 0:HB])
            nc.sync.dma_start(wB[:, HB:2 * HB], WB_d[:, HB:2 * HB])
            nc.sync.dma_start(aB[0][:, HB:2 * HB], AB_d[:, HB:2 * HB])
            wvB = pool.tile([128, KC * OL], DT, tag="WVB", name="wvB")
            nc.sync.dma_start(wvB[:], WVB_d[:, :])
            for n in range(1, 4):
                nc.sync.dma_start(aB[n][:],
                                  AB_d[:, n * KC * CQ:(n + 1) * KC * CQ])
            wpB = pool.tile([128, 2 * C], DT, tag="WPB", name="wpB")
            nc.sync.dma_start(wpB[:], WPB_d[:, :])

            # ---- persistent intermediates (qt/kt per 512-col chunk so the
            # interleaved attention never waits on unrelated chunk writes)
            qt_sb = [[pool.tile([128, CQ], DT, tag=f"QT{i}_{n}",
                                name=f"qt{i}_{n}") for n in range(4)]
                     for i in range(2)]
            kt_sb = [[pool.tile([128, CQ], DT, tag=f"KT{i}_{n}",
                                name=f"kt{i}_{n}") for n in range(4)]
                     for i in range(2)]
            v_sb = [pool.tile([128, H_LOC * (D + 1)], DT, tag=f"V{i}",
                              name=f"v{i}") for i in range(NT)]
            ao_sb = [pool.tile([128, T], DT, tag=f"AO{i}", name=f"ao{i}")
                     for i in range(2)]

            # ones columns for the softmax denominators: one strided memset
            # per V tile, done up front off the critical path
            for tt in range(NT):
                nc.vector.memset(
                    v_sb[tt].rearrange("p (h x) -> p h x", x=D + 1)[:, :, D:D + 1],
                    1.0)
            # warm the GpSimd ucode paths and preload the ACT exp table so
            # neither cold-start lands mid-attention
            warm = wpool.tile([128, 8], f32, tag="warm")
            nc.vector.memset(warm[:], 1.0)
            nc.gpsimd.affine_select(
                out=warm[:], in_=warm[:],
                compare_op=mybir.AluOpType.is_ge, fill=0.0, base=0,
                pattern=[[1, 8]], channel_multiplier=-1)
            warm2 = wpool.tile([128, 8], f32, tag="warm2")
            nc.gpsimd.partition_broadcast(warm2[:], warm[0:1, :])
            warm3 = wpool.tile([128, 8], f32, tag="warm3")
            nc.scalar.activation(warm3[:], warm2[:], Exp, scale=0.001)
            # dummy matmuls during the ~15us input-DMA window: ~5us of PE
            # activity flips the HAM clock gate to 8/8 so the first QKV
            # chains run at 2.4GHz instead of 1.2
            wmm = wpool.tile([128, CQ], DT, tag="wmm")
            nc.vector.memset(wmm[:], 0.0)
            for _ in range(4):
                pw = psum.tile([128, CQ], f32, tag="prj", bufs=2, name="pw")
                for k in range(8):
                    nc.tensor.matmul(pw[:, 0:128], wmm[:, 0:128],
                                     wmm[:, 0:128],
                                     start=(k == 0), stop=(k == 7))

            # ---- QKV / V projection chain emitters (one chain each) ----
            def emit_qk_chain(n, m):
                ps = psum.tile([128, CQ], f32, tag="prj", bufs=2, name="ps")
                for kc in range(KC):
                    nc.tensor.matmul(
                        ps[:],
                        wB[:, kc * CQ + m * 128:kc * CQ + (m + 1) * 128],
                        aB[n][:, kc * CQ:(kc + 1) * CQ],
                        start=(kc == 0), stop=(kc == KC - 1))
                if m < 2:
                    nc.vector.tensor_scalar_add(
                        qt_sb[m][n][:], ps[:], bq2[:, m:m + 1])
                else:
                    nc.vector.tensor_copy(kt_sb[m - 2][n][:], ps[:])

            def emit_v_chain(tt):
                ps = psum.tile([128, CQ], f32, tag="prj", bufs=2,
                               name="psv")[:, 0:OL]
                for kc in range(KC):
                    nc.tensor.matmul(
                        ps[:],
                        aB[tt // 4][:, kc * CQ + (tt % 4) * 128:
                                    kc * CQ + (tt % 4 + 1) * 128],
                        wvB[:, kc * OL:(kc + 1) * OL],
                        start=(kc == 0), stop=(kc == KC - 1))
                nc.vector.tensor_copy(
                    v_sb[tt].rearrange("p (h x) -> p h x", x=D + 1)[:, :, 0:D],
                    ps.rearrange("p (h x) -> p h x", x=D))

            yt_tiles = {}

            def emit_proj_chain(tt, n2):
                if n2 == 0:
                    yt_tiles[tt] = wpool.tile([128, C], DT, tag="y", bufs=3,
                                              name="yt")
                yt = yt_tiles[tt]
                ps = psum.tile([128, CQ], f32, tag="prj", bufs=2, name="psp")
                for kc2 in range(2):
                    nc.tensor.matmul(
                        ps[:],
                        ao_sb[kc2][:, tt * 128:(tt + 1) * 128],
                        wpB[:, kc2 * C + n2 * CQ:kc2 * C + (n2 + 1) * CQ],
                        start=(kc2 == 0), stop=(kc2 == 1))
                nc.vector.tensor_copy(yt[:, n2 * CQ:(n2 + 1) * CQ], ps[:])
                if n2 == 1:
                    del yt_tiles[tt]
                    nc.sync.dma_start(Y[tt * 128:(tt + 1) * 128, :], yt[:])

            # ---- head-pair flash attention, software-pipelined ----
            blocks = []
            group_start = []
            fi = 0
            for qj in range(4):
                group_start.append(fi)
                for hp in range(2):
                    n_kc = (qj + 1) * 4
                    order = list(range(qj * 4, n_kc)) + list(range(0, qj * 4))
                    blocks.append((qj, hp, order))
                    fi += len(order)
            flat = [(bi, j) for bi, (_, _, order) in enumerate(blocks)
                    for j in range(len(order))]
            # Later groups' QKV/V chains AND the previous group's proj
            # chains are spread one-at-a-time through the attention stream:
            # the PE absorbs them between S/PV pairs (keeping its HAM clock
            # warm) while the scalar engine grinds exps. QKV chains for
            # group g must be emitted before the S-emitter (LOOKAHEAD
            # ahead) reaches group_start[g].
            inject = {
                0: [(emit_qk_chain, (1, 0)), (emit_qk_chain, (1, 1))],
                1: [(emit_qk_chain, (1, 2)), (emit_qk_chain, (1, 3))],
                2: [(emit_v_chain, (4,)), (emit_v_chain, (5,))],
                3: [(emit_v_chain, (6,)), (emit_v_chain, (7,))],
                # group 1 (flat 8..23) hosts QKV/V(2) + proj(0)
                8: [(emit_qk_chain, (2, 0))],
                9: [(emit_proj_chain, (0, 0))],
                10: [(emit_qk_chain, (2, 1))],
                11: [(emit_proj_chain, (0, 1))],
                12: [(emit_qk_chain, (2, 2))],
                13: [(emit_proj_chain, (1, 0))],
                14: [(emit_qk_chain, (2, 3))],
                15: [(emit_proj_chain, (1, 1))],
                16: [(emit_v_chain, (8,))],
                17: [(emit_proj_chain, (2, 0))],
                18: [(emit_v_chain, (9,))],
                19: [(emit_proj_chain, (2, 1))],
                20: [(emit_v_chain, (10,))],
                21: [(emit_v_chain, (11,))],
                22: [(emit_proj_chain, (3, 0))],
                23: [(emit_proj_chain, (3, 1))],
                # group 2 (flat 24..47) hosts QKV/V(3) + proj(1)
                25: [(emit_proj_chain, (4, 0))],
                26: [(emit_qk_chain, (3, 0))],
                28: [(emit_proj_chain, (4, 1))],
                29: [(emit_qk_chain, (3, 1))],
                31: [(emit_proj_chain, (5, 0))],
                32: [(emit_qk_chain, (3, 2))],
                34: [(emit_proj_chain, (5, 1))],
                35: [(emit_qk_chain, (3, 3))],
                37: [(emit_proj_chain, (6, 0))],
                38: [(emit_v_chain, (12,))],
                39: [(emit_proj_chain, (6, 1))],
                40: [(emit_v_chain, (13,))],
                42: [(emit_v_chain, (14,))],
                43: [(emit_proj_chain, (7, 0))],
                44: [(emit_v_chain, (15,))],
                46: [(emit_proj_chain, (7, 1))],
                # group 3 (flat 48..79) hosts proj(2)
                50: [(emit_proj_chain, (8, 0))],
                53: [(emit_proj_chain, (8, 1))],
                56: [(emit_proj_chain, (9, 0))],
                59: [(emit_proj_chain, (9, 1))],
                62: [(emit_proj_chain, (10, 0))],
                65: [(emit_proj_chain, (10, 1))],
                68: [(emit_proj_chain, (11, 0))],
                71: [(emit_proj_chain, (11, 1))],
            }
            LOOKAHEAD = 3
            pv_tiles = {}
            pts = {}

            # group 0's QKV / V chains must precede the pipeline seed
            for m in range(4):
                emit_qk_chain(0, m)
            for tt in range(4):
                emit_v_chain(tt)

            def emit_s(idx):
                bi, j = flat[idx]
                qj, hp, order = blocks[bi]
                kc = order[j]
                q0 = qj * CQ
                qoff = max(0, kc * CK - q0)   # causal trim: 0/128/256/384
                width = CQ - qoff
                sp = psum.tile([128, 2 * CQ], f32, tag="sp", bufs=2,
                               name="sp")
                # the two heads use disjoint PE row groups -> concurrent
                nc.tensor.matmul(
                    sp[:, 0:width],
                    kt_sb[hp][kc // 4][0:D, (kc % 4) * 128:(kc % 4 + 1) * 128],
                    qt_sb[hp][qj][0:D, qoff:CQ],
                    start=True, stop=True)
                nc.tensor.matmul(
                    sp[:, CQ:CQ + width],
                    kt_sb[hp][kc // 4][D:128, (kc % 4) * 128:(kc % 4 + 1) * 128],
                    qt_sb[hp][qj][D:128, qoff:CQ],
                    start=True, stop=True)
                pt = wpool.tile([128, 2 * CQ], DT, tag="pT", bufs=6)
                sview = sp.rearrange("p (two q) -> p two q", two=2)[:, :, 0:width]
                pview = pt.rearrange("p (two q) -> p two q", two=2)[:, :, 0:width]
                nc.scalar.activation(pview, sview, Exp, scale=1.0 / 8.0)
                if kc >= qj * 4:   # diagonal chunk: mask q < k on both heads
                    nc.gpsimd.affine_select(
                        out=pview, in_=pview,
                        compare_op=mybir.AluOpType.is_ge,
                        fill=0.0, base=0,
                        pattern=[[0, 2], [1, width]], channel_multiplier=-1)
                pts[(bi, kc)] = (pt, qoff, width)

            for idx in range(min(LOOKAHEAD, len(flat))):
                emit_s(idx)
            for i, (bi, j) in enumerate(flat):
                qj, hp, order = blocks[bi]
                kc = order[j]
                n_kc = len(order)
                q0 = qj * CQ
                if j == 0:
                    pv_tiles[bi] = psum.tile([D + 1, 2 * CQ], f32, tag="pv",
                                             bufs=1, name="pv")
                pv = pv_tiles[bi]
                pt, qoff, width = pts.pop((bi, kc))
                nc.tensor.matmul(
                    pv[:, qoff:qoff + width],
                    v_sb[kc][:, (2 * hp) * (D + 1):(2 * hp + 1) * (D + 1)],
                    pt[:, 0:width],
                    start=(j == 0), stop=(j == n_kc - 1))
                nc.tensor.matmul(
                    pv[:, CQ + qoff:CQ + qoff + width],
                    v_sb[kc][:, (2 * hp + 1) * (D + 1):(2 * hp + 2) * (D + 1)],
                    pt[:, CQ:CQ + width],
                    start=(j == 0), stop=(j == n_kc - 1))
                for fn, args in inject.get(i, ()):
                    fn(*args)
                if i + LOOKAHEAD < len(flat):
                    emit_s(i + LOOKAHEAD)
                if j != n_kc - 1:
                    continue
                # block complete: evacuate PV fast (pvs on DVE, the ones-row
                # on the idle-ish scalar engine) so the single pv PSUM slot
                # unlocks for the next block, then recip/broadcast/scale.
                del pv_tiles[bi]
                last_block = (hp == 1 and qj == 3)
                if not last_block:
                    pvs = wpool.tile([D, 2 * CQ], f32, tag="pvs", bufs=2)
                    nc.vector.tensor_copy(pvs[:], pv[0:D, :])
                    ls = wpool.tile([1, 2 * CQ], f32, tag="ls", bufs=2)
                    nc.scalar.copy(ls[:], pv[D:D + 1, :])
                    r = wpool.tile([1, 2 * CQ], f32, tag="r", bufs=2)
                    with nc.allow_low_precision(reason="softmax denom"):
                        # approx_fast needs SBUF input at partition base 0
                        nc.vector.reciprocal_approx_fast(r[:], ls[:])
                    rbs = wpool.tile([D, 2 * CQ], f32, tag="rbs", bufs=2)
                    nc.gpsimd.partition_broadcast(rbs[:], r[:])
                    nc.vector.tensor_mul(
                        ao_sb[hp][0:D, q0:q0 + CQ], pvs[:, 0:CQ],
                        rbs[:, 0:CQ])
                    nc.vector.tensor_mul(
                        ao_sb[hp][D:128, q0:q0 + CQ], pvs[:, CQ:2 * CQ],
                        rbs[:, CQ:2 * CQ])
                    continue
                # last block: its finalize chain is fully exposed in the
                # tail, so pipeline it per 512-half and keep the PE's HAM
                # clock warm with dummy matmuls until the proj burst.
                for half in range(2):
                    hs = slice(half * CQ, (half + 1) * CQ)
                    ls = wpool.tile([1, CQ], f32, tag="ls2", bufs=2)
                    nc.scalar.copy(ls[:], pv[D:D + 1, hs])
                    r = wpool.tile([1, CQ], f32, tag="r2", bufs=2)
                    with nc.allow_low_precision(reason="softmax denom"):
                        nc.vector.reciprocal_approx_fast(r[:], ls[:])
                    rbs = wpool.tile([D, CQ], f32, tag="rbs2", bufs=2)
                    nc.gpsimd.partition_broadcast(rbs[:], r[:])
                    nc.vector.tensor_mul(
                        ao_sb[hp][half * D:(half + 1) * D, q0:q0 + CQ],
                        pv[0:D, hs], rbs[:])
                sp_w = psum.tile([128, 2 * CQ], f32, tag="sp", bufs=2,
                                 name="spw")
                for k in range(12):
                    nc.tensor.matmul(sp_w[:, 0:CQ], wmm[:, 0:128], wmm[:],
                                     start=(k == 0), stop=(k == 11))
                for tt in range(12, 16):
                    emit_proj_chain(tt, 0)
                    emit_proj_chain(tt, 1)

    nc.compile()
    return nc


def _get_compiled():
    global _COMPILED
    if _COMPILED is None:
        _COMPILED = _build()
    return _COMPILED


def make_in_maps(x, Wq, bq, Wk, Wv, Wp):
    in_maps = []
    for c in range(N_CORES):
        b, g = divmod(c, 4)
        sl = slice(g * OL, (g + 1) * OL)
        XT = np.ascontiguousarray(x[b].T)                      # [C, T]
        AB = (XT.reshape(KC, 128, 4, CQ).transpose(1, 2, 0, 3)
              .reshape(128, 4 * KC * CQ))
        WQK = np.concatenate([Wq[sl].T, Wk[sl].T], axis=1)     # [C, 512]
        WB = WQK.reshape(KC, 128, CQ).transpose(1, 0, 2).reshape(128, KC * CQ)
        WVB = (Wv[sl].T.reshape(KC, 128, OL).transpose(1, 0, 2)
               .reshape(128, KC * OL))
        WPB = (Wp[:, sl].T.reshape(2, 128, C).transpose(1, 0, 2)
               .reshape(128, 2 * C))
        in_maps.append({
            "AB": np.ascontiguousarray(AB).astype(NP_DT),
            "WB": np.ascontiguousarray(WB).astype(NP_DT),
            "WVB": np.ascontiguousarray(WVB).astype(NP_DT),
            "WPB": np.ascontiguousarray(WPB).astype(NP_DT),
            "BQ": np.ascontiguousarray(bq[sl].reshape(2, 128).T).astype(
                np.float32),
        })
    return in_maps


_RUNNER = None


def _make_runner():
    """Build the 8-core shard_map executable once (run_bass_via_pjrt re-jits
    on every call; this caches the traced/compiled callable)."""
    import jax
    from jax.sharding import Mesh, PartitionSpec
    from jax.experimental.shard_map import shard_map
    import concourse.mybir as mybir_
    from concourse import bass2jax

    nc = _get_compiled()
    bass2jax.install_neuronx_cc_hook()

    partition_name = (nc.partition_id_tensor.name
                      if nc.partition_id_tensor else None)
    in_names, out_names, out_avals, zero_outs = [], [], [], []
    for alloc in nc.m.functions[0].allocations:
        if not isinstance(alloc, mybir_.MemoryLocationSet):
            continue
        name = alloc.memorylocations[0].name
        if alloc.kind == "ExternalInput":
            if name != partition_name:
                in_names.append(name)
        elif alloc.kind == "ExternalOutput":
            shape = tuple(alloc.tensor_shape)
            dtype = mybir_.dt.np(alloc.dtype)
            out_names.append(name)
            out_avals.append(jax.core.ShapedArray(shape, dtype))
            zero_outs.append(np.zeros(shape, dtype))
    n_params = len(in_names)
    n_outs = len(out_avals)
    all_in_names = list(in_names) + list(out_names)
    if partition_name is not None:
        all_in_names.append(partition_name)
    donate = tuple(range(n_params, n_params + n_outs))

    def _body(*args):
        operands = list(args)
        if partition_name is not None:
            operands.append(bass2jax.partition_id_tensor())
        outs = bass2jax._bass_exec_p.bind(
            *operands,
            out_avals=tuple(out_avals),
            in_names=tuple(all_in_names),
            out_names=tuple(out_names),
            lowering_input_output_aliases=(),
            sim_require_finite=True,
            sim_require_nnan=True,
            nc=nc,
        )
        return tuple(outs)

    devices = jax.devices()[:N_CORES]
    mesh = Mesh(np.asarray(devices), ("core",))
    in_specs = (PartitionSpec("core"),) * (n_params + n_outs)
    out_specs = (PartitionSpec("core"),) * n_outs
    sharded = jax.jit(
        shard_map(_body, mesh=mesh, in_specs=in_specs, out_specs=out_specs,
                  check_rep=False),
        donate_argnums=donate, keep_unused=True)

    def run(in_maps):
        per_core = [[np.asarray(m[name]) for name in in_names]
                    for m in in_maps]
        concat_in = [
            np.concatenate([per_core[c][i] for c in range(N_CORES)], axis=0)
            for i in range(n_params)]
        concat_zeros = [
            np.zeros((N_CORES * z.shape[0], *z.shape[1:]), z.dtype)
            for z in zero_outs]
        out_arrs = sharded(*concat_in, *concat_zeros)
        return [
            {name: np.asarray(out_arrs[i]).reshape(
                N_CORES, *out_avals[i].shape)[c]
             for i, name in enumerate(out_names)}
            for c in range(N_CORES)]

    return run


def _get_runner():
    global _RUNNER
    if _RUNNER is None:
        _RUNNER = _make_runner()
    return _RUNNER


def _axon_reset():
    try:
        import ctypes
        lib = ctypes.CDLL("/opt/axon/libaxon_pjrt.so")
        if hasattr(lib, "axon_reset"):
            lib.axon_reset()
    except Exception:
        pass


def kernel(x, Wq, bq, Wk, bk, Wv, bv, Wp, bp):
    x = np.asarray(x, dtype=np.float32)
    Wq = np.asarray(Wq, dtype=np.float32)
    bq = np.asarray(bq, dtype=np.float32)
    Wk = np.asarray(Wk, dtype=np.float32)
    Wv = np.asarray(Wv, dtype=np.float32)
    Wp = np.asarray(Wp, dtype=np.float32)
    bv = np.asarray(bv, dtype=np.float32)
    bp = np.asarray(bp, dtype=np.float32)

    in_maps = make_in_maps(x, Wq, bq, Wk, Wv, Wp)

    results = None
    for attempt in range(3):
        try:
            results = _get_runner()(in_maps)
            break
        except Exception:
            if attempt == 2:
                raise
            _axon_reset()  # recover a wedged accelerator and retry

    extra = bv @ Wp.T + bp  # bv/bp fold out of the device kernel
    out = np.empty((B, T, C), dtype=np.float32)
    for b in range(B):
        acc = results[4 * b]["Y"].astype(np.float32)
        for g in range(1, 4):
            acc = acc + results[4 * b + g]["Y"].astype(np.float32)
        out[b] = acc + extra
    return out


# revision 25
# speedup vs baseline: 1.4433x; 1.0092x over previous
"""Multi-head causal attention (B=2, T=2048, C=1024, H=16, D=64) on 8 trn2 cores.

Sharding: core c -> batch b = c//4, head group g = c%4 (4 heads each),
Megatron-style: QKV column-parallel, proj row-parallel. Partial outputs are
summed on the host; bk is softmax-invariant and dropped, bv/bp fold into a
host-side constant. All matmul operands are bf16 (host-cast), accumulation
and softmax stay fp32.

Device kernel (per core):
  All inputs are host-packed into a handful of big partition-major tiles so
  the whole load is 8 dma_starts (each costs ~680ns serialized on the sync
  engine - per-slice loads would gate the kernel on sync for ~50us).
  Q^T (+bq) / K^T = W.T @ A        [256, 2048]   channels on partitions
  V = A.T @ Wv_loc.T               [2048, 4*(64+1)]  natural layout, a ones
                                   column per head for softmax denominators
  Attention runs on HEAD PAIRS (heads 2hp, 2hp+1 live on partition halves
  0:64 / 64:128 of the qt/kt tiles):
     S^T even/odd = K_h^T.T @ Q_h^T   two K=64 matmuls into adjacent PSUM
                                      banks; disjoint PE row-groups (0,0) /
                                      (64,0) -> they run CONCURRENTLY
     P^T pair = exp(0.125*S^T)        ONE ACT call over both banks (N=2*w)
                                      amortizing the ~352-cycle ACT overhead
     diagonal chunks masked on GpSimd via one paired affine_select
     PV~[65, 2*512] += V~_h.T @ P^T   row 64 accumulates the denominator l
     out^T = PV[0:64] * bcast(1/l)    approx-recip + paired broadcast,
                                      multiplied straight out of PSUM
  The QKV/V projection chains are SPREAD one-at-a-time through the attention
  stream, so the PE never idles (and its HAM clock stays warm) while the
  scalar engine grinds exps.
  Y = attn-out^T.T @ Wp_loc.T      [2048, 1024]  bf16 partial, proj
                                   interleaved per q-chunk, summed on host
"""

import sys

sys.path.insert(0, "/opt/trn_rl_repo")

import numpy as np
import ml_dtypes

NP_DT = ml_dtypes.bfloat16

import concourse.bass as bass  # noqa: F401
import concourse.mybir as mybir
import concourse.tile as tile
from concourse import bacc
from concourse.bass_utils import run_bass_kernel_spmd

N_CORES = 8
B, T, C = 2, 2048, 1024
H, D = 16, 64
H_LOC = 4              # heads per core
OL = H_LOC * D         # local channels = 256
CQ = 512               # PSUM-bank q chunk
CK = 128               # k chunk (partition dim)
NT = T // 128          # 16
KC = C // 128          # 8 contraction chunks for QKV

f32 = mybir.dt.float32
bf16 = mybir.dt.bfloat16
DT = bf16  # matmul operand dtype

_COMPILED = None


def _build():
    nc = bacc.Bacc("TRN2", debug=False, num_devices=N_CORES)

    # host-packed partition-major inputs (see make_in_maps)
    AB_d = nc.dram_tensor("AB", [128, 4 * KC * CQ], DT, kind="ExternalInput").ap()
    WB_d = nc.dram_tensor("WB", [128, KC * CQ], DT, kind="ExternalInput").ap()
    WVB_d = nc.dram_tensor("WVB", [128, KC * OL], DT, kind="ExternalInput").ap()
    WPB_d = nc.dram_tensor("WPB", [128, 2 * C], DT, kind="ExternalInput").ap()
    BQ_d = nc.dram_tensor("BQ", [128, 2], f32, kind="ExternalInput").ap()
    Y = nc.dram_tensor("Y", [T, C], DT, kind="ExternalOutput").ap()

    Exp = mybir.ActivationFunctionType.Exp

    with tile.TileContext(nc) as tc:
        with tc.tile_pool(name="sbuf", bufs=1) as pool, \
             tc.tile_pool(name="work", bufs=1) as wpool, \
             tc.tile_pool(name="psum", bufs=1, space="PSUM") as psum:

            # ---- resident inputs: 8 dma_starts in consumption order ----
            bq2 = pool.tile([128, 2], f32, tag="BQ", name="bq2")
            nc.sync.dma_start(bq2[:], BQ_d[:, :])
            # wB/aB0 land in kc-halves so the first QK chain's early kc
            # matmuls can start ~4.5us before the full tiles arrive
            HB = KC * CQ // 2
            wB = pool.tile([128, KC * CQ], DT, tag="WB", name="wB")
            aB = [pool.tile([128, KC * CQ], DT, tag=f"AB{n}", name=f"aB{n}")
                  for n in range(4)]
            nc.sync.dma_start(wB[:, 0:HB], WB_d[:, 0:HB])
            nc.sync.dma_start(aB[0][:, 0:HB], AB_d[:, 0:HB])
            nc.sync.dma_start(wB[:, HB:2 * HB], WB_d[:, HB:2 * HB])
            nc.sync.dma_start(aB[0][:, HB:2 * HB], AB_d[:, HB:2 * HB])
            wvB = pool.tile([128, KC * OL], DT, tag="WVB", name="wvB")
            nc.sync.dma_start(wvB[:], WVB_d[:, :])
            for n in range(1, 4):
                nc.sync.dma_start(aB[n][:],
                                  AB_d[:, n * KC * CQ:(n + 1) * KC * CQ])
            wpB = pool.tile([128, 2 * C], DT, tag="WPB", name="wpB")
            nc.sync.dma_start(wpB[:], WPB_d[:, :])

            # ---- persistent intermediates (qt/kt per 512-col chunk so the
            # interleaved attention never waits on unrelated chunk writes)
            qt_sb = [[pool.tile([128, CQ], DT, tag=f"QT{i}_{n}",
                                name=f"qt{i}_{n}") for n in range(4)]
                     for i in range(2)]
            kt_sb = [[pool.tile([128, CQ], DT, tag=f"KT{i}_{n}",
                                name=f"kt{i}_{n}") for n in range(4)]
                     for i in range(2)]
            v_sb = [pool.tile([128, H_LOC * (D + 1)], DT, tag=f"V{i}",
                              name=f"v{i}") for i in range(NT)]
            ao_sb = [pool.tile([128, T], DT, tag=f"AO{i}", name=f"ao{i}")
                     for i in range(2)]

            # ones columns for the softmax denominators: one strided memset
            # per V tile, done up front off the critical path
            for tt in range(NT):
                nc.vector.memset(
                    v_sb[tt].rearrange("p (h x) -> p h x", x=D + 1)[:, :, D:D + 1],
                    1.0)
            # warm the GpSimd ucode paths and preload the ACT exp table so
            # neither cold-start lands mid-attention
            warm = wpool.tile([128, 8], f32, tag="warm")
            nc.vector.memset(warm[:], 1.0)
            nc.gpsimd.affine_select(
                out=warm[:], in_=warm[:],
                compare_op=mybir.AluOpType.is_ge, fill=0.0, base=0,
                pattern=[[1, 8]], channel_multiplier=-1)
            warm2 = wpool.tile([128, 8], f32, tag="warm2")
            nc.gpsimd.partition_broadcast(warm2[:], warm[0:1, :])
            warm3 = wpool.tile([128, 8], f32, tag="warm3")
            nc.scalar.activation(warm3[:], warm2[:], Exp, scale=0.001)
            # dummy matmuls during the ~15us input-DMA window: ~5us of PE
            # activity flips the HAM clock gate to 8/8 so the first QKV
            # chains run at 2.4GHz instead of 1.2
            wmm = wpool.tile([128, CQ], DT, tag="wmm")
            nc.vector.memset(wmm[:], 0.0)
            for _ in range(5):
                pw = psum.tile([128, CQ], f32, tag="prj", bufs=2, name="pw")
                for k in range(8):
                    nc.tensor.matmul(pw[:, 0:128], wmm[:, 0:128],
                                     wmm[:, 0:128],
                                     start=(k == 0), stop=(k == 7))

            # ---- QKV / V projection chain emitters (one chain each) ----
            def emit_qk_chain(n, m):
                ps = psum.tile([128, CQ], f32, tag="prj", bufs=2, name="ps")
                for kc in range(KC):
                    nc.tensor.matmul(
                        ps[:],
                        wB[:, kc * CQ + m * 128:kc * CQ + (m + 1) * 128],
                        aB[n][:, kc * CQ:(kc + 1) * CQ],
                        start=(kc == 0), stop=(kc == KC - 1))
                if m < 2:
                    nc.vector.tensor_scalar_add(
                        qt_sb[m][n][:], ps[:], bq2[:, m:m + 1])
                else:
                    nc.vector.tensor_copy(kt_sb[m - 2][n][:], ps[:])

            def emit_v_chain(tt):
                ps = psum.tile([128, CQ], f32, tag="prj", bufs=2,
                               name="psv")[:, 0:OL]
                for kc in range(KC):
                    nc.tensor.matmul(
                        ps[:],
                        aB[tt // 4][:, kc * CQ + (tt % 4) * 128:
                                    kc * CQ + (tt % 4 + 1) * 128],
                        wvB[:, kc * OL:(kc + 1) * OL],
                        start=(kc == 0), stop=(kc == KC - 1))
                nc.vector.tensor_copy(
                    v_sb[tt].rearrange("p (h x) -> p h x", x=D + 1)[:, :, 0:D],
                    ps.rearrange("p (h x) -> p h x", x=D))

            yt_tiles = {}

            def emit_proj_chain(tt, n2):
                if n2 == 0:
                    yt_tiles[tt] = wpool.tile([128, C], DT, tag="y", bufs=3,
                                              name="yt")
                yt = yt_tiles[tt]
                ps = psum.tile([128, CQ], f32, tag="prj", bufs=2, name="psp")
                for kc2 in range(2):
                    nc.tensor.matmul(
                        ps[:],
                        ao_sb[kc2][:, tt * 128:(tt + 1) * 128],
                        wpB[:, kc2 * C + n2 * CQ:kc2 * C + (n2 + 1) * CQ],
                        start=(kc2 == 0), stop=(kc2 == 1))
                nc.vector.tensor_copy(yt[:, n2 * CQ:(n2 + 1) * CQ], ps[:])
                if n2 == 1:
                    del yt_tiles[tt]
                    nc.sync.dma_start(Y[tt * 128:(tt + 1) * 128, :], yt[:])

            # ---- head-pair flash attention, software-pipelined ----
            blocks = []
            group_start = []
            fi = 0
            for qj in range(4):
                group_start.append(fi)
                for hp in range(2):
                    n_kc = (qj + 1) * 4
                    order = list(range(qj * 4, n_kc)) + list(range(0, qj * 4))
                    blocks.append((qj, hp, order))
                    fi += len(order)
            flat = [(bi, j) for bi, (_, _, order) in enumerate(blocks)
                    for j in range(len(order))]
            # Later groups' QKV/V chains AND the previous group's proj
            # chains are spread one-at-a-time through the attention stream:
            # the PE absorbs them between S/PV pairs (keeping its HAM clock
            # warm) while the scalar engine grinds exps. QKV chains for
            # group g must be emitted before the S-emitter (LOOKAHEAD
            # ahead) reaches group_start[g].
            inject = {
                0: [(emit_qk_chain, (1, 0)), (emit_qk_chain, (1, 1))],
                1: [(emit_qk_chain, (1, 2)), (emit_qk_chain, (1, 3))],
                2: [(emit_v_chain, (4,)), (emit_v_chain, (5,))],
                3: [(emit_v_chain, (6,)), (emit_v_chain, (7,))],
                # group 1 (flat 8..23) hosts QKV/V(2) + proj(0)
                8: [(emit_qk_chain, (2, 0))],
                9: [(emit_proj_chain, (0, 0))],
                10: [(emit_qk_chain, (2, 1))],
                11: [(emit_proj_chain, (0, 1))],
                12: [(emit_qk_chain, (2, 2))],
                13: [(emit_proj_chain, (1, 0))],
                14: [(emit_qk_chain, (2, 3))],
                15: [(emit_proj_chain, (1, 1))],
                16: [(emit_v_chain, (8,))],
                17: [(emit_proj_chain, (2, 0))],
                18: [(emit_v_chain, (9,))],
                19: [(emit_proj_chain, (2, 1))],
                20: [(emit_v_chain, (10,))],
                21: [(emit_v_chain, (11,))],
                22: [(emit_proj_chain, (3, 0))],
                23: [(emit_proj_chain, (3, 1))],
                # group 2 (flat 24..47) hosts QKV/V(3) + proj(1)
                25: [(emit_proj_chain, (4, 0))],
                26: [(emit_qk_chain, (3, 0))],
                28: [(emit_proj_chain, (4, 1))],
                29: [(emit_qk_chain, (3, 1))],
                31: [(emit_proj_chain, (5, 0))],
                32: [(emit_qk_chain, (3, 2))],
                34: [(emit_proj_chain, (5, 1))],
                35: [(emit_qk_chain, (3, 3))],
                37: [(emit_proj_chain, (6, 0))],
                38: [(emit_v_chain, (12,))],
                39: [(emit_proj_chain, (6, 1))],
                40: [(emit_v_chain, (13,))],
                42: [(emit_v_chain, (14,))],
                43: [(emit_proj_chain, (7, 0))],
                44: [(emit_v_chain, (15,))],
                46: [(emit_proj_chain, (7, 1))],
                # group 3 (flat 48..79) hosts proj(2)
                50: [(emit_proj_chain, (8, 0))],
                53: [(emit_proj_chain, (8, 1))],
                56: [(emit_proj_chain, (9, 0))],
                59: [(emit_proj_chain, (9, 1))],
                62: [(emit_proj_chain, (10, 0))],
                65: [(emit_proj_chain, (10, 1))],
                68: [(emit_proj_chain, (11, 0))],
                71: [(emit_proj_chain, (11, 1))],
            }
            LOOKAHEAD = 3
            pv_tiles = {}
            pts = {}

            # group 0's QKV / V chains must precede the pipeline seed
            for m in range(4):
                emit_qk_chain(0, m)
            for tt in range(4):
                emit_v_chain(tt)

            def emit_s(idx):
                bi, j = flat[idx]
                qj, hp, order = blocks[bi]
                kc = order[j]
                q0 = qj * CQ
                qoff = max(0, kc * CK - q0)   # causal trim: 0/128/256/384
                width = CQ - qoff
                sp = psum.tile([128, 2 * CQ], f32, tag="sp", bufs=2,
                               name="sp")
                # the two heads use disjoint PE row groups -> concurrent
                nc.tensor.matmul(
                    sp[:, 0:width],
                    kt_sb[hp][kc // 4][0:D, (kc % 4) * 128:(kc % 4 + 1) * 128],
                    qt_sb[hp][qj][0:D, qoff:CQ],
                    start=True, stop=True)
                nc.tensor.matmul(
                    sp[:, CQ:CQ + width],
                    kt_sb[hp][kc // 4][D:128, (kc % 4) * 128:(kc % 4 + 1) * 128],
                    qt_sb[hp][qj][D:128, qoff:CQ],
                    start=True, stop=True)
                pt = wpool.tile([128, 2 * CQ], DT, tag="pT", bufs=6)
                sview = sp.rearrange("p (two q) -> p two q", two=2)[:, :, 0:width]
                pview = pt.rearrange("p (two q) -> p two q", two=2)[:, :, 0:width]
                nc.scalar.activation(pview, sview, Exp, scale=1.0 / 8.0)
                if kc >= qj * 4:   # diagonal chunk: mask q < k on both heads
                    nc.gpsimd.affine_select(
                        out=pview, in_=pview,
                        compare_op=mybir.AluOpType.is_ge,
                        fill=0.0, base=0,
                        pattern=[[0, 2], [1, width]], channel_multiplier=-1)
                pts[(bi, kc)] = (pt, qoff, width)

            for idx in range(min(LOOKAHEAD, len(flat))):
                emit_s(idx)
            for i, (bi, j) in enumerate(flat):
                qj, hp, order = blocks[bi]
                kc = order[j]
                n_kc = len(order)
                q0 = qj * CQ
                if j == 0:
                    pv_tiles[bi] = psum.tile([D + 1, 2 * CQ], f32, tag="pv",
                                             bufs=1, name="pv")
                pv = pv_tiles[bi]
                pt, qoff, width = pts.pop((bi, kc))
                nc.tensor.matmul(
                    pv[:, qoff:qoff + width],
                    v_sb[kc][:, (2 * hp) * (D + 1):(2 * hp + 1) * (D + 1)],
                    pt[:, 0:width],
                    start=(j == 0), stop=(j == n_kc - 1))
                nc.tensor.matmul(
                    pv[:, CQ + qoff:CQ + qoff + width],
                    v_sb[kc][:, (2 * hp + 1) * (D + 1):(2 * hp + 2) * (D + 1)],
                    pt[:, CQ:CQ + width],
                    start=(j == 0), stop=(j == n_kc - 1))
                for fn, args in inject.get(i, ()):
                    fn(*args)
                if i + LOOKAHEAD < len(flat):
                    emit_s(i + LOOKAHEAD)
                if j != n_kc - 1:
                    continue
                # block complete: evacuate PV fast (pvs on DVE, the ones-row
                # on the idle-ish scalar engine) so the single pv PSUM slot
                # unlocks for the next block, then recip/broadcast/scale.
                del pv_tiles[bi]
                pvs = wpool.tile([D, 2 * CQ], f32, tag="pvs", bufs=2)
                nc.vector.tensor_copy(pvs[:], pv[0:D, :])
                ls = wpool.tile([1, 2 * CQ], f32, tag="ls", bufs=2)
                nc.scalar.copy(ls[:], pv[D:D + 1, :])
                r = wpool.tile([1, 2 * CQ], f32, tag="r", bufs=2)
                with nc.allow_low_precision(reason="softmax denom"):
                    # approx_fast needs SBUF input at partition base 0
                    nc.vector.reciprocal_approx_fast(r[:], ls[:])
                rbs = wpool.tile([D, 2 * CQ], f32, tag="rbs", bufs=2)
                nc.gpsimd.partition_broadcast(rbs[:], r[:])
                nc.vector.tensor_mul(
                    ao_sb[hp][0:D, q0:q0 + CQ], pvs[:, 0:CQ], rbs[:, 0:CQ])
                nc.vector.tensor_mul(
                    ao_sb[hp][D:128, q0:q0 + CQ], pvs[:, CQ:2 * CQ],
                    rbs[:, CQ:2 * CQ])
                if hp == 1 and qj == 3:
                    # last block: keep the PE's HAM clock warm with dummy
                    # matmuls while its finalize chain drains, then proj.
                    sp_w = psum.tile([128, 2 * CQ], f32, tag="sp", bufs=2,
                                     name="spw")
                    for k in range(24):
                        nc.tensor.matmul(sp_w[:, 0:CQ], wmm[:, 0:128],
                                         wmm[:],
                                         start=(k == 0), stop=(k == 23))
                    for tt in range(12, 16):
                        emit_proj_chain(tt, 0)
                        emit_proj_chain(tt, 1)

    nc.compile()
    return nc


def _get_compiled():
    global _COMPILED
    if _COMPILED is None:
        _COMPILED = _build()
    return _COMPILED


def make_in_maps(x, Wq, bq, Wk, Wv, Wp):
    in_maps = []
    for c in range(N_CORES):
        b, g = divmod(c, 4)
        sl = slice(g * OL, (g + 1) * OL)
        XT = np.ascontiguousarray(x[b].T)                      # [C, T]
        AB = (XT.reshape(KC, 128, 4, CQ).transpose(1, 2, 0, 3)
              .reshape(128, 4 * KC * CQ))
        WQK = np.concatenate([Wq[sl].T, Wk[sl].T], axis=1)     # [C, 512]
        WB = WQK.reshape(KC, 128, CQ).transpose(1, 0, 2).reshape(128, KC * CQ)
        WVB = (Wv[sl].T.reshape(KC, 128, OL).transpose(1, 0, 2)
               .reshape(128, KC * OL))
        WPB = (Wp[:, sl].T.reshape(2, 128, C).transpose(1, 0, 2)
               .reshape(128, 2 * C))
        in_maps.append({
            "AB": np.ascontiguousarray(AB).astype(NP_DT),
            "WB": np.ascontiguousarray(WB).astype(NP_DT),
            "WVB": np.ascontiguousarray(WVB).astype(NP_DT),
            "WPB": np.ascontiguousarray(WPB).astype(NP_DT),
            "BQ": np.ascontiguousarray(bq[sl].reshape(2, 128).T).astype(
                np.float32),
        })
    return in_maps


_RUNNER = None


def _make_runner():
    """Build the 8-core shard_map executable once (run_bass_via_pjrt re-jits
    on every call; this caches the traced/compiled callable)."""
    import jax
    from jax.sharding import Mesh, PartitionSpec
    from jax.experimental.shard_map import shard_map
    import concourse.mybir as mybir_
    from concourse import bass2jax

    nc = _get_compiled()
    bass2jax.install_neuronx_cc_hook()

    partition_name = (nc.partition_id_tensor.name
                      if nc.partition_id_tensor else None)
    in_names, out_names, out_avals, zero_outs = [], [], [], []
    for alloc in nc.m.functions[0].allocations:
        if not isinstance(alloc, mybir_.MemoryLocationSet):
            continue
        name = alloc.memorylocations[0].name
        if alloc.kind == "ExternalInput":
            if name != partition_name:
                in_names.append(name)
        elif alloc.kind == "ExternalOutput":
            shape = tuple(alloc.tensor_shape)
            dtype = mybir_.dt.np(alloc.dtype)
            out_names.append(name)
            out_avals.append(jax.core.ShapedArray(shape, dtype))
            zero_outs.append(np.zeros(shape, dtype))
    n_params = len(in_names)
    n_outs = len(out_avals)
    all_in_names = list(in_names) + list(out_names)
    if partition_name is not None:
        all_in_names.append(partition_name)
    donate = tuple(range(n_params, n_params + n_outs))

    def _body(*args):
        operands = list(args)
        if partition_name is not None:
            operands.append(bass2jax.partition_id_tensor())
        outs = bass2jax._bass_exec_p.bind(
            *operands,
            out_avals=tuple(out_avals),
            in_names=tuple(all_in_names),
            out_names=tuple(out_names),
            lowering_input_output_aliases=(),
            sim_require_finite=True,
            sim_require_nnan=True,
            nc=nc,
        )
        return tuple(outs)

    devices = jax.devices()[:N_CORES]
    mesh = Mesh(np.asarray(devices), ("core",))
    in_specs = (PartitionSpec("core"),) * (n_params + n_outs)
    out_specs = (PartitionSpec("core"),) * n_outs
    sharded = jax.jit(
        shard_map(_body, mesh=mesh, in_specs=in_specs, out_specs=out_specs,
                  check_rep=False),
        donate_argnums=donate, keep_unused=True)

    def run(in_maps):
        per_core = [[np.asarray(m[name]) for name in in_names]
                    for m in in_maps]
        concat_in = [
            np.concatenate([per_core[c][i] for c in range(N_CORES)], axis=0)
            for i in range(n_params)]
        concat_zeros = [
            np.zeros((N_CORES * z.shape[0], *z.shape[1:]), z.dtype)
            for z in zero_outs]
        out_arrs = sharded(*concat_in, *concat_zeros)
        return [
            {name: np.asarray(out_arrs[i]).reshape(
                N_CORES, *out_avals[i].shape)[c]
             for i, name in enumerate(out_names)}
            for c in range(N_CORES)]

    return run


def _get_runner():
    global _RUNNER
    if _RUNNER is None:
        _RUNNER = _make_runner()
    return _RUNNER


def _axon_reset():
    try:
        import ctypes
        lib = ctypes.CDLL("/opt/axon/libaxon_pjrt.so")
        if hasattr(lib, "axon_reset"):
            lib.axon_reset()
    except Exception:
        pass


def kernel(x, Wq, bq, Wk, bk, Wv, bv, Wp, bp):
    x = np.asarray(x, dtype=np.float32)
    Wq = np.asarray(Wq, dtype=np.float32)
    bq = np.asarray(bq, dtype=np.float32)
    Wk = np.asarray(Wk, dtype=np.float32)
    Wv = np.asarray(Wv, dtype=np.float32)
    Wp = np.asarray(Wp, dtype=np.float32)
    bv = np.asarray(bv, dtype=np.float32)
    bp = np.asarray(bp, dtype=np.float32)

    in_maps = make_in_maps(x, Wq, bq, Wk, Wv, Wp)

    results = None
    for attempt in range(3):
        try:
            results = _get_runner()(in_maps)
            break
        except Exception:
            if attempt == 2:
                raise
            _axon_reset()  # recover a wedged accelerator and retry

    extra = bv @ Wp.T + bp  # bv/bp fold out of the device kernel
    out = np.empty((B, T, C), dtype=np.float32)
    for b in range(B):
        acc = results[4 * b]["Y"].astype(np.float32)
        for g in range(1, 4):
            acc = acc + results[4 * b + g]["Y"].astype(np.float32)
        out[b] = acc + extra
    return out


# revision 29
# speedup vs baseline: 1.5451x; 1.0705x over previous
"""Multi-head causal attention (B=2, T=2048, C=1024, H=16, D=64) on 8 trn2 cores.

Sharding: core c -> batch b = c//4, head group g = c%4 (4 heads each),
Megatron-style: QKV column-parallel, proj row-parallel. Partial outputs are
summed on the host; bk is softmax-invariant and dropped, bv/bp fold into a
host-side constant. All matmul operands are bf16 (host-cast), accumulation
and softmax stay fp32.

Device kernel (per core):
  All inputs are host-packed into a handful of big partition-major tiles so
  the whole load is 8 dma_starts (each costs ~680ns serialized on the sync
  engine - per-slice loads would gate the kernel on sync for ~50us).
  Q^T (+bq) / K^T = W.T @ A        [256, 2048]   channels on partitions
  V = A.T @ Wv_loc.T               [2048, 4*(64+1)]  natural layout, a ones
                                   column per head for softmax denominators
  Attention runs on HEAD PAIRS (heads 2hp, 2hp+1 live on partition halves
  0:64 / 64:128 of the qt/kt tiles):
     S^T even/odd = K_h^T.T @ Q_h^T   two K=64 matmuls into adjacent PSUM
                                      banks; disjoint PE row-groups (0,0) /
                                      (64,0) -> they run CONCURRENTLY
     P^T pair = exp(0.125*S^T)        ONE ACT call over both banks (N=2*w)
                                      amortizing the ~352-cycle ACT overhead
     diagonal chunks masked on GpSimd via one paired affine_select
     PV~[65, 2*512] += V~_h.T @ P^T   row 64 accumulates the denominator l
     out^T = PV[0:64] * bcast(1/l)    approx-recip + paired broadcast,
                                      multiplied straight out of PSUM
  The QKV/V projection chains are SPREAD one-at-a-time through the attention
  stream, so the PE never idles (and its HAM clock stays warm) while the
  scalar engine grinds exps.
  Y = attn-out^T.T @ Wp_loc.T      [2048, 1024]  bf16 partial, proj
                                   interleaved per q-chunk, summed on host
"""

import sys

sys.path.insert(0, "/opt/trn_rl_repo")

import numpy as np
import ml_dtypes

NP_DT = ml_dtypes.bfloat16

import concourse.bass as bass  # noqa: F401
import concourse.mybir as mybir
import concourse.tile as tile
from concourse import bacc
from concourse.bass_utils import run_bass_kernel_spmd

N_CORES = 8
B, T, C = 2, 2048, 1024
H, D = 16, 64
H_LOC = 4              # heads per core
OL = H_LOC * D         # local channels = 256
CQ = 512               # PSUM-bank q chunk
CK = 128               # k chunk (partition dim)
NT = T // 128          # 16
KC = C // 128          # 8 contraction chunks for QKV

f32 = mybir.dt.float32
bf16 = mybir.dt.bfloat16
DT = bf16  # matmul operand dtype

_COMPILED = None


def _build():
    nc = bacc.Bacc("TRN2", debug=False, num_devices=N_CORES)

    # host-packed partition-major inputs (see make_in_maps)
    AB_d = nc.dram_tensor("AB", [128, 4 * KC * CQ], DT, kind="ExternalInput").ap()
    WB_d = nc.dram_tensor("WB", [128, KC * CQ], DT, kind="ExternalInput").ap()
    WVB_d = nc.dram_tensor("WVB", [128, KC * OL], DT, kind="ExternalInput").ap()
    WPB_d = nc.dram_tensor("WPB", [128, 2 * C], DT, kind="ExternalInput").ap()
    BQ_d = nc.dram_tensor("BQ", [128, 2], f32, kind="ExternalInput").ap()
    Y = nc.dram_tensor("Y", [T, C], DT, kind="ExternalOutput").ap()

    Exp = mybir.ActivationFunctionType.Exp

    with tile.TileContext(nc) as tc:
        with tc.tile_pool(name="sbuf", bufs=1) as pool, \
             tc.tile_pool(name="work", bufs=1) as wpool, \
             tc.tile_pool(name="psum", bufs=1, space="PSUM") as psum:

            # ---- resident inputs: 8 dma_starts in consumption order ----
            bq2 = pool.tile([128, 2], f32, tag="BQ", name="bq2")
            nc.sync.dma_start(bq2[:], BQ_d[:, :])
            # wB/aB0 land in kc-halves so the first QK chain's early kc
            # matmuls can start ~4.5us before the full tiles arrive
            HB = KC * CQ // 2
            wB = pool.tile([128, KC * CQ], DT, tag="WB", name="wB")
            aB = [pool.tile([128, KC * CQ], DT, tag=f"AB{n}", name=f"aB{n}")
                  for n in range(4)]
            nc.sync.dma_start(wB[:, 0:HB], WB_d[:, 0:HB])
            nc.sync.dma_start(aB[0][:, 0:HB], AB_d[:, 0:HB])
            nc.sync.dma_start(wB[:, HB:2 * HB], WB_d[:, HB:2 * HB])
            nc.sync.dma_start(aB[0][:, HB:2 * HB], AB_d[:, HB:2 * HB])
            wvB = pool.tile([128, KC * OL], DT, tag="WVB", name="wvB")
            nc.sync.dma_start(wvB[:], WVB_d[:, :])
            for n in range(1, 4):
                nc.sync.dma_start(aB[n][:],
                                  AB_d[:, n * KC * CQ:(n + 1) * KC * CQ])
            wpB = pool.tile([128, 2 * C], DT, tag="WPB", name="wpB")
            nc.sync.dma_start(wpB[:], WPB_d[:, :])

            # ---- persistent intermediates (qt/kt per 512-col chunk so the
            # interleaved attention never waits on unrelated chunk writes)
            qt_sb = [[pool.tile([128, CQ], DT, tag=f"QT{i}_{n}",
                                name=f"qt{i}_{n}") for n in range(4)]
                     for i in range(2)]
            kt_sb = [[pool.tile([128, CQ], DT, tag=f"KT{i}_{n}",
                                name=f"kt{i}_{n}") for n in range(4)]
                     for i in range(2)]
            v_sb = [pool.tile([128, H_LOC * (D + 1)], DT, tag=f"V{i}",
                              name=f"v{i}") for i in range(NT)]
            ao_sb = [pool.tile([128, T], DT, tag=f"AO{i}", name=f"ao{i}")
                     for i in range(2)]

            # ones columns for the softmax denominators: one strided memset
            # per V tile, done up front off the critical path
            for tt in range(NT):
                nc.vector.memset(
                    v_sb[tt].rearrange("p (h x) -> p h x", x=D + 1)[:, :, D:D + 1],
                    1.0)
            # warm the GpSimd ucode paths and preload the ACT exp table so
            # neither cold-start lands mid-attention
            warm = wpool.tile([128, 8], f32, tag="warm")
            nc.vector.memset(warm[:], 1.0)
            nc.gpsimd.affine_select(
                out=warm[:], in_=warm[:],
                compare_op=mybir.AluOpType.is_ge, fill=0.0, base=0,
                pattern=[[1, 8]], channel_multiplier=-1)
            warm2 = wpool.tile([128, 8], f32, tag="warm2")
            nc.gpsimd.partition_broadcast(warm2[:], warm[0:1, :])
            warm3 = wpool.tile([128, 8], f32, tag="warm3")
            nc.scalar.activation(warm3[:], warm2[:], Exp, scale=0.001)
            # dummy matmuls during the ~15us input-DMA window: ~5us of PE
            # activity flips the HAM clock gate to 8/8 so the first QKV
            # chains run at 2.4GHz instead of 1.2
            wmm = wpool.tile([128, CQ], DT, tag="wmm")
            nc.vector.memset(wmm[:], 0.0)
            for _ in range(5):
                pw = psum.tile([128, CQ], f32, tag="prj", bufs=2, name="pw")
                for k in range(8):
                    nc.tensor.matmul(pw[:, 0:128], wmm[:, 0:128],
                                     wmm[:, 0:128],
                                     start=(k == 0), stop=(k == 7))

            # ---- QKV / V projection chain emitters (one chain each) ----
            def emit_qk_chain(n, m):
                ps = psum.tile([128, CQ], f32, tag="prj", bufs=2, name="ps")
                for kc in range(KC):
                    nc.tensor.matmul(
                        ps[:],
                        wB[:, kc * CQ + m * 128:kc * CQ + (m + 1) * 128],
                        aB[n][:, kc * CQ:(kc + 1) * CQ],
                        start=(kc == 0), stop=(kc == KC - 1))
                if m < 2:
                    nc.vector.tensor_scalar_add(
                        qt_sb[m][n][:], ps[:], bq2[:, m:m + 1])
                else:
                    nc.vector.tensor_copy(kt_sb[m - 2][n][:], ps[:])

            def emit_v_chain(tt):
                ps = psum.tile([128, CQ], f32, tag="prj", bufs=2,
                               name="psv")[:, 0:OL]
                for kc in range(KC):
                    nc.tensor.matmul(
                        ps[:],
                        aB[tt // 4][:, kc * CQ + (tt % 4) * 128:
                                    kc * CQ + (tt % 4 + 1) * 128],
                        wvB[:, kc * OL:(kc + 1) * OL],
                        start=(kc == 0), stop=(kc == KC - 1))
                nc.vector.tensor_copy(
                    v_sb[tt].rearrange("p (h x) -> p h x", x=D + 1)[:, :, 0:D],
                    ps.rearrange("p (h x) -> p h x", x=D))

            yt_tiles = {}

            def emit_proj_chain(tt, n2):
                if n2 == 0:
                    yt_tiles[tt] = wpool.tile([128, C], DT, tag="y", bufs=3,
                                              name="yt")
                yt = yt_tiles[tt]
                ps = psum.tile([128, CQ], f32, tag="prj", bufs=2, name="psp")
                for kc2 in range(2):
                    nc.tensor.matmul(
                        ps[:],
                        ao_sb[kc2][:, tt * 128:(tt + 1) * 128],
                        wpB[:, kc2 * C + n2 * CQ:kc2 * C + (n2 + 1) * CQ],
                        start=(kc2 == 0), stop=(kc2 == 1))
                nc.vector.tensor_copy(yt[:, n2 * CQ:(n2 + 1) * CQ], ps[:])
                if n2 == 1:
                    del yt_tiles[tt]
                    nc.sync.dma_start(Y[tt * 128:(tt + 1) * 128, :], yt[:])

            # ---- head-pair flash attention, software-pipelined ----
            blocks = []
            group_start = []
            fi = 0
            for qj in range(4):
                group_start.append(fi)
                for hp in range(2):
                    n_kc = (qj + 1) * 4
                    order = list(range(qj * 4, n_kc)) + list(range(0, qj * 4))
                    blocks.append((qj, hp, order))
                    fi += len(order)
            flat = [(bi, j) for bi, (_, _, order) in enumerate(blocks)
                    for j in range(len(order))]
            # Later groups' QKV/V chains AND the previous group's proj
            # chains are spread one-at-a-time through the attention stream:
            # the PE absorbs them between S/PV pairs (keeping its HAM clock
            # warm) while the scalar engine grinds exps. QKV chains for
            # group g must be emitted before the S-emitter (LOOKAHEAD
            # ahead) reaches group_start[g].
            inject = {
                0: [(emit_qk_chain, (1, 0)), (emit_qk_chain, (1, 1))],
                1: [(emit_qk_chain, (1, 2)), (emit_qk_chain, (1, 3))],
                2: [(emit_v_chain, (4,)), (emit_v_chain, (5,))],
                3: [(emit_v_chain, (6,)), (emit_v_chain, (7,))],
                # group 1 (flat 8..23) hosts QKV/V(2)
                8: [(emit_qk_chain, (2, 0))],
                10: [(emit_qk_chain, (2, 1))],
                12: [(emit_qk_chain, (2, 2))],
                14: [(emit_qk_chain, (2, 3))],
                16: [(emit_v_chain, (8,))],
                18: [(emit_v_chain, (9,))],
                20: [(emit_v_chain, (10,))],
                21: [(emit_v_chain, (11,))],
                # group 2 (flat 24..47) hosts QKV/V(3) + proj(0)
                25: [(emit_proj_chain, (0, 0))],
                26: [(emit_qk_chain, (3, 0))],
                28: [(emit_proj_chain, (0, 1))],
                29: [(emit_qk_chain, (3, 1))],
                31: [(emit_proj_chain, (1, 0))],
                32: [(emit_qk_chain, (3, 2))],
                34: [(emit_proj_chain, (1, 1))],
                35: [(emit_qk_chain, (3, 3))],
                37: [(emit_proj_chain, (2, 0))],
                38: [(emit_v_chain, (12,))],
                39: [(emit_proj_chain, (2, 1))],
                40: [(emit_v_chain, (13,))],
                42: [(emit_v_chain, (14,))],
                43: [(emit_proj_chain, (3, 0))],
                44: [(emit_v_chain, (15,))],
                46: [(emit_proj_chain, (3, 1))],
                # group 3 (flat 48..79, ACT-heavy) hosts proj(1) + proj(2)
                49: [(emit_proj_chain, (4, 0))],
                51: [(emit_proj_chain, (4, 1))],
                53: [(emit_proj_chain, (5, 0))],
                55: [(emit_proj_chain, (5, 1))],
                57: [(emit_proj_chain, (6, 0))],
                59: [(emit_proj_chain, (6, 1))],
                61: [(emit_proj_chain, (7, 0))],
                63: [(emit_proj_chain, (7, 1))],
                64: [(emit_proj_chain, (8, 0))],
                66: [(emit_proj_chain, (8, 1))],
                68: [(emit_proj_chain, (9, 0))],
                70: [(emit_proj_chain, (9, 1))],
                72: [(emit_proj_chain, (10, 0))],
                74: [(emit_proj_chain, (10, 1))],
                76: [(emit_proj_chain, (11, 0))],
                78: [(emit_proj_chain, (11, 1))],
            }
            LOOKAHEAD = 3
            pv_tiles = {}
            pts = {}

            # Upfront chains: block (0,0) only needs the m=0/m=2 chains, so
            # emit those first and seed the S pipeline immediately — the
            # scalar engine starts on exps ~5us earlier.
            emit_qk_chain(0, 0)
            emit_qk_chain(0, 2)

            def emit_s(idx):
                bi, j = flat[idx]
                qj, hp, order = blocks[bi]
                kc = order[j]
                q0 = qj * CQ
                qoff = max(0, kc * CK - q0)   # causal trim: 0/128/256/384
                width = CQ - qoff
                sp = psum.tile([128, 2 * CQ], f32, tag="sp", bufs=2,
                               name="sp")
                # the two heads use disjoint PE row groups -> concurrent
                nc.tensor.matmul(
                    sp[:, 0:width],
                    kt_sb[hp][kc // 4][0:D, (kc % 4) * 128:(kc % 4 + 1) * 128],
                    qt_sb[hp][qj][0:D, qoff:CQ],
                    start=True, stop=True)
                nc.tensor.matmul(
                    sp[:, CQ:CQ + width],
                    kt_sb[hp][kc // 4][D:128, (kc % 4) * 128:(kc % 4 + 1) * 128],
                    qt_sb[hp][qj][D:128, qoff:CQ],
                    start=True, stop=True)
                pt = wpool.tile([128, 2 * CQ], DT, tag="pT", bufs=6)
                sview = sp.rearrange("p (two q) -> p two q", two=2)[:, :, 0:width]
                pview = pt.rearrange("p (two q) -> p two q", two=2)[:, :, 0:width]
                nc.scalar.activation(pview, sview, Exp, scale=1.0 / 8.0)
                if kc >= qj * 4:   # diagonal chunk: mask q < k on both heads
                    nc.gpsimd.affine_select(
                        out=pview, in_=pview,
                        compare_op=mybir.AluOpType.is_ge,
                        fill=0.0, base=0,
                        pattern=[[0, 2], [1, width]], channel_multiplier=-1)
                pts[(bi, kc)] = (pt, qoff, width)

            for idx in range(min(LOOKAHEAD, len(flat))):
                emit_s(idx)
            emit_qk_chain(0, 1)
            emit_qk_chain(0, 3)
            for tt in range(4):
                emit_v_chain(tt)
            for i, (bi, j) in enumerate(flat):
                qj, hp, order = blocks[bi]
                kc = order[j]
                n_kc = len(order)
                q0 = qj * CQ
                if j == 0:
                    pv_tiles[bi] = psum.tile([D + 1, 2 * CQ], f32, tag="pv",
                                             bufs=1, name="pv")
                pv = pv_tiles[bi]
                pt, qoff, width = pts.pop((bi, kc))
                nc.tensor.matmul(
                    pv[:, qoff:qoff + width],
                    v_sb[kc][:, (2 * hp) * (D + 1):(2 * hp + 1) * (D + 1)],
                    pt[:, 0:width],
                    start=(j == 0), stop=(j == n_kc - 1))
                nc.tensor.matmul(
                    pv[:, CQ + qoff:CQ + qoff + width],
                    v_sb[kc][:, (2 * hp + 1) * (D + 1):(2 * hp + 2) * (D + 1)],
                    pt[:, CQ:CQ + width],
                    start=(j == 0), stop=(j == n_kc - 1))
                for fn, args in inject.get(i, ()):
                    fn(*args)
                if i + LOOKAHEAD < len(flat):
                    emit_s(i + LOOKAHEAD)
                if j != n_kc - 1:
                    continue
                # block complete: evacuate PV fast (pvs on DVE, the ones-row
                # on the idle-ish scalar engine) so the single pv PSUM slot
                # unlocks for the next block, then recip/broadcast/scale.
                del pv_tiles[bi]
                pvs = wpool.tile([D, 2 * CQ], f32, tag="pvs", bufs=2)
                nc.vector.tensor_copy(pvs[:], pv[0:D, :])
                ls = wpool.tile([1, 2 * CQ], f32, tag="ls", bufs=2)
                if qj == 3:
                    # group 3 is ACT-bound: keep the ones-row copy off the
                    # scalar engine there
                    nc.vector.tensor_copy(ls[:], pv[D:D + 1, :])
                else:
                    nc.scalar.copy(ls[:], pv[D:D + 1, :])
                r = wpool.tile([1, 2 * CQ], f32, tag="r", bufs=2)
                with nc.allow_low_precision(reason="softmax denom"):
                    # approx_fast needs SBUF input at partition base 0
                    nc.vector.reciprocal_approx_fast(r[:], ls[:])
                rbs = wpool.tile([D, 2 * CQ], f32, tag="rbs", bufs=2)
                nc.gpsimd.partition_broadcast(rbs[:], r[:])
                nc.vector.tensor_mul(
                    ao_sb[hp][0:D, q0:q0 + CQ], pvs[:, 0:CQ], rbs[:, 0:CQ])
                nc.vector.tensor_mul(
                    ao_sb[hp][D:128, q0:q0 + CQ], pvs[:, CQ:2 * CQ],
                    rbs[:, CQ:2 * CQ])
                if hp == 1 and qj == 3:
                    # last block: keep the PE's HAM clock warm with dummy
                    # matmuls while its finalize chain drains, then proj.
                    sp_w = psum.tile([128, 2 * CQ], f32, tag="sp", bufs=2,
                                     name="spw")
                    for k in range(24):
                        nc.tensor.matmul(sp_w[:, 0:CQ], wmm[:, 0:128],
                                         wmm[:],
                                         start=(k == 0), stop=(k == 23))
                    for tt in range(12, 16):
                        emit_proj_chain(tt, 0)
                        emit_proj_chain(tt, 1)

    nc.compile()
    return nc


def _get_compiled():
    global _COMPILED
    if _COMPILED is None:
        _COMPILED = _build()
    return _COMPILED


def make_in_maps(x, Wq, bq, Wk, Wv, Wp):
    in_maps = []
    for c in range(N_CORES):
        b, g = divmod(c, 4)
        sl = slice(g * OL, (g + 1) * OL)
        XT = np.ascontiguousarray(x[b].T)                      # [C, T]
        AB = (XT.reshape(KC, 128, 4, CQ).transpose(1, 2, 0, 3)
              .reshape(128, 4 * KC * CQ))
        WQK = np.concatenate([Wq[sl].T, Wk[sl].T], axis=1)     # [C, 512]
        WB = WQK.reshape(KC, 128, CQ).transpose(1, 0, 2).reshape(128, KC * CQ)
        WVB = (Wv[sl].T.reshape(KC, 128, OL).transpose(1, 0, 2)
               .reshape(128, KC * OL))
        WPB = (Wp[:, sl].T.reshape(2, 128, C).transpose(1, 0, 2)
               .reshape(128, 2 * C))
        in_maps.append({
            "AB": np.ascontiguousarray(AB).astype(NP_DT),
            "WB": np.ascontiguousarray(WB).astype(NP_DT),
            "WVB": np.ascontiguousarray(WVB).astype(NP_DT),
            "WPB": np.ascontiguousarray(WPB).astype(NP_DT),
            "BQ": np.ascontiguousarray(bq[sl].reshape(2, 128).T).astype(
                np.float32),
        })
    return in_maps


_RUNNER = None


def _make_runner():
    """Build the 8-core shard_map executable once (run_bass_via_pjrt re-jits
    on every call; this caches the traced/compiled callable)."""
    import jax
    from jax.sharding import Mesh, PartitionSpec
    from jax.experimental.shard_map import shard_map
    import concourse.mybir as mybir_
    from concourse import bass2jax

    nc = _get_compiled()
    bass2jax.install_neuronx_cc_hook()

    partition_name = (nc.partition_id_tensor.name
                      if nc.partition_id_tensor else None)
    in_names, out_names, out_avals, zero_outs = [], [], [], []
    for alloc in nc.m.functions[0].allocations:
        if not isinstance(alloc, mybir_.MemoryLocationSet):
            continue
        name = alloc.memorylocations[0].name
        if alloc.kind == "ExternalInput":
            if name != partition_name:
                in_names.append(name)
        elif alloc.kind == "ExternalOutput":
            shape = tuple(alloc.tensor_shape)
            dtype = mybir_.dt.np(alloc.dtype)
            out_names.append(name)
            out_avals.append(jax.core.ShapedArray(shape, dtype))
            zero_outs.append(np.zeros(shape, dtype))
    n_params = len(in_names)
    n_outs = len(out_avals)
    all_in_names = list(in_names) + list(out_names)
    if partition_name is not None:
        all_in_names.append(partition_name)
    donate = tuple(range(n_params, n_params + n_outs))

    def _body(*args):
        operands = list(args)
        if partition_name is not None:
            operands.append(bass2jax.partition_id_tensor())
        outs = bass2jax._bass_exec_p.bind(
            *operands,
            out_avals=tuple(out_avals),
            in_names=tuple(all_in_names),
            out_names=tuple(out_names),
            lowering_input_output_aliases=(),
            sim_require_finite=True,
            sim_require_nnan=True,
            nc=nc,
        )
        return tuple(outs)

    devices = jax.devices()[:N_CORES]
    mesh = Mesh(np.asarray(devices), ("core",))
    in_specs = (PartitionSpec("core"),) * (n_params + n_outs)
    out_specs = (PartitionSpec("core"),) * n_outs
    sharded = jax.jit(
        shard_map(_body, mesh=mesh, in_specs=in_specs, out_specs=out_specs,
                  check_rep=False),
        donate_argnums=donate, keep_unused=True)

    def run(in_maps):
        per_core = [[np.asarray(m[name]) for name in in_names]
                    for m in in_maps]
        concat_in = [
            np.concatenate([per_core[c][i] for c in range(N_CORES)], axis=0)
            for i in range(n_params)]
        concat_zeros = [
            np.zeros((N_CORES * z.shape[0], *z.shape[1:]), z.dtype)
            for z in zero_outs]
        out_arrs = sharded(*concat_in, *concat_zeros)
        return [
            {name: np.asarray(out_arrs[i]).reshape(
                N_CORES, *out_avals[i].shape)[c]
             for i, name in enumerate(out_names)}
            for c in range(N_CORES)]

    return run


def _get_runner():
    global _RUNNER
    if _RUNNER is None:
        _RUNNER = _make_runner()
    return _RUNNER


def _axon_reset():
    try:
        import ctypes
        lib = ctypes.CDLL("/opt/axon/libaxon_pjrt.so")
        if hasattr(lib, "axon_reset"):
            lib.axon_reset()
    except Exception:
        pass


def kernel(x, Wq, bq, Wk, bk, Wv, bv, Wp, bp):
    x = np.asarray(x, dtype=np.float32)
    Wq = np.asarray(Wq, dtype=np.float32)
    bq = np.asarray(bq, dtype=np.float32)
    Wk = np.asarray(Wk, dtype=np.float32)
    Wv = np.asarray(Wv, dtype=np.float32)
    Wp = np.asarray(Wp, dtype=np.float32)
    bv = np.asarray(bv, dtype=np.float32)
    bp = np.asarray(bp, dtype=np.float32)

    in_maps = make_in_maps(x, Wq, bq, Wk, Wv, Wp)

    results = None
    for attempt in range(3):
        try:
            results = _get_runner()(in_maps)
            break
        except Exception:
            if attempt == 2:
                raise
            _axon_reset()  # recover a wedged accelerator and retry

    extra = bv @ Wp.T + bp  # bv/bp fold out of the device kernel
    out = np.empty((B, T, C), dtype=np.float32)
    for b in range(B):
        acc = results[4 * b]["Y"].astype(np.float32)
        for g in range(1, 4):
            acc = acc + results[4 * b + g]["Y"].astype(np.float32)
        out[b] = acc + extra
    return out
